# revision 1
# baseline (speedup 1.0000x reference)
"""Trainium2 Bass kernel for GQA attention prefill (B=2, T=2048, D=4096, N=32, K=8, H=128).

Sharding: 8 cores = 2 (batch) x 4 (head-groups). Each core handles one batch
element, 8 q-heads and its 2 kv-heads, producing a partial output projection
(summed over its heads, bf16). Host sums the 4 partials per batch element.

Per-core pipeline, software-pipelined per head so PE never idles:
  passA(tb):  k,v projections from x_sb (x DMA'd once per t-block, kept
              resident in SBUF); rope(k) via a +-1 permutation matmul plus
              DVE elementwise with host cos/sin tables.
  per head h: q-projection matmuls for head h+1 are emitted interleaved with
              head h's attention s-block loop, so PE has dense independent
              work while ACT computes exp. Attention: logitsT [s128,t<=512]
              = kT-block @ qt (bf16, no transposes), exp on ACT, 0/1
              triangle mask multiply on DVE for diagonal tiles only, AV
              accumulates in PSUM. Softmax denominators: non-diagonal exp
              tiles accumulate on the Pool engine (off the critical path);
              one ones-vector matmul over that accumulator plus tiny per-tile
              ones-matmuls for the 4 diagonal blocks finish l on PE.
              During the last head, the next t-block's x / cos / sin DMAs
              are emitted so passA never waits on the queue.
  ph3(tb):    output projection from per-head enc tiles (no whole-tile dep),
              PSUM -> bf16 SBUF copies round-robined over DVE/Pool/ACT,
              DMA out per 512-wide d-chunk.
"""

import os
import sys

import numpy as np

for _p in ("/opt/trn_rl_repo", "/root/.axon_site/_ro/trn_rl_repo"):
    if _p not in sys.path and os.path.isdir(_p):
        sys.path.append(_p)

import ml_dtypes

BF16 = ml_dtypes.bfloat16

P = 128
T = 2048
D = 4096
H = 128
NQ = 8   # q heads per core
NKV = 2  # kv heads per core
TB = 512
NTB = T // TB        # 4
DT = D // P          # 32 d-tiles
NSB = T // P         # 16 s-blocks
TC = TB // P         # 4 t-chunks per t-block
NDC = D // TB        # 8 d-chunks for the output projection
SCALE = float(H) ** -0.5

_STATE = {}


def _build_nc():
    import concourse.mybir as mybir
    import concourse.tile as tile
    from concourse import bacc
    from concourse import bass_isa

    f32 = mybir.dt.float32
    f32r = mybir.dt.float32r
    bf16 = mybir.dt.bfloat16
    Alu = mybir.AluOpType
    Act = mybir.ActivationFunctionType

    nc = bacc.Bacc(None, target_bir_lowering=False, debug=False)

    xT = nc.dram_tensor("xT", [D, T], bf16, kind="ExternalInput")
    wq = nc.dram_tensor("wq", [D, NQ, H], bf16, kind="ExternalInput")
    wk = nc.dram_tensor("wk", [D, NKV, H], bf16, kind="ExternalInput")
    wv = nc.dram_tensor("wv", [D, NKV, H], bf16, kind="ExternalInput")
    wo = nc.dram_tensor("wo", [NQ, H, D], bf16, kind="ExternalInput")
    cos = nc.dram_tensor("cos", [P, T], bf16, kind="ExternalInput")
    sin = nc.dram_tensor("sin", [P, T], bf16, kind="ExternalInput")
    mrot = nc.dram_tensor("mrot", [P, P], bf16, kind="ExternalInput")
    tri = nc.dram_tensor("tri", [P, P], bf16, kind="ExternalInput")
    ones = nc.dram_tensor("ones", [P, 1], f32r, kind="ExternalInput")
    onesb = nc.dram_tensor("onesb", [P, 1], bf16, kind="ExternalInput")
    y = nc.dram_tensor("y", [T, D], bf16, kind="ExternalOutput")

    with tile.TileContext(nc) as tc:
        with (
            tc.tile_pool(name="const", bufs=1) as const,
            tc.tile_pool(name="xp", bufs=2) as xp,
            tc.tile_pool(name="wqp", bufs=2) as wqp,
            tc.tile_pool(name="qtp", bufs=3) as qtp,
            tc.tile_pool(name="rp", bufs=3) as rp,
            tc.tile_pool(name="ep", bufs=6) as ep,
            tc.tile_pool(name="eap", bufs=2) as eap,
            tc.tile_pool(name="encp", bufs=9) as encp,
            tc.tile_pool(name="lp", bufs=1) as lp,
            tc.tile_pool(name="wop", bufs=2) as wop,
            tc.tile_pool(name="yp", bufs=2) as yp,
            tc.tile_pool(name="ps", bufs=1, space="PSUM") as ps,
        ):
            wk_sb = const.tile([P, DT, NKV, H], bf16, tag="wk")
            wv_sb = const.tile([P, DT, NKV * H], bf16, tag="wv")
            wk_r = wk.rearrange("(dt p) h e -> p dt h e", p=P)
            wv_r = wv.rearrange("(dt p) h e -> p dt (h e)", p=P)
            tri_sb = const.tile([P, P], bf16, tag="tri")
            cos_sb = const.tile([P, T], bf16, tag="cos")
            sin_sb = const.tile([P, T], bf16, tag="sin")
            kT_all = const.tile([P, NKV, T], bf16, tag="kT")
            v_all = const.tile([P, NKV, NSB, H], bf16, tag="v")

            def x_dma_closures(tb, x_sb, tables=True, nch=8):
                """nch closures, each DMA-ing a DT/nch-d-tile chunk of x for
                tb (plus this t-block's cos/sin on the first chunk)."""
                tsl = slice(tb * TB, (tb + 1) * TB)
                step = DT // nch

                def mk(c8):
                    def emit():
                        nc.sync.dma_start(
                            x_sb[:, c8 * step:(c8 + 1) * step, :],
                            xT[c8 * step * P:(c8 + 1) * step * P, tsl]
                            .rearrange("(g p) t -> p g t", p=P))
                        if c8 == 0 and tables:
                            nc.sync.dma_start(cos_sb[:, tsl], cos[:, tsl])
                            nc.sync.dma_start(sin_sb[:, tsl], sin[:, tsl])
                    return emit

                return [mk(c8) for c8 in range(nch)]

            def rope(dst, src_ps, tb):
                """dst[:] = rope(src_ps) for one head's [H, TB] block.
                Half-swap via SBUF->SBUF DMA partition reorder (no PE);
                the sign of the swapped half is folded into the sin table
                ([-sin; +sin])."""
                cs = cos_sb[:, tb * TB:(tb + 1) * TB]
                sn = sin_sb[:, tb * TB:(tb + 1) * TB]
                raw = rp.tile([P, TB], bf16, tag="raw", name="raw")
                nc.scalar.copy(raw[:], src_ps[:])
                shuf = rp.tile([P, TB], bf16, tag="shuf", name="shuf")
                nc.sync.dma_start(shuf[0:P // 2, :], raw[P // 2:P, :])
                nc.sync.dma_start(shuf[P // 2:P, :], raw[0:P // 2, :])
                tmp = rp.tile([P, TB], bf16, tag="tmp", name="tmp")
                nc.vector.tensor_tensor(dst, raw[:], cs, Alu.mult)
                nc.vector.tensor_tensor(tmp[:], shuf[:], sn, Alu.mult)
                nc.vector.tensor_tensor(dst, dst, tmp[:], Alu.add)

            def kv_k_chunks(tbx):
                """k-projection matmul chunk closures (+ psk tiles) for tbx."""
                x_sb = x_tiles[tbx]
                psk = [ps.tile([P, TB], f32, tag="big", bufs=6,
                               name=f"psk{_k}") for _k in range(NKV)]

                def mk(d0, d1):
                    def emit():
                        for dt in range(d0, d1):
                            for kk in range(NKV):
                                nc.tensor.matmul(
                                    psk[kk][:], wk_sb[:, dt, kk, :],
                                    x_sb[:, dt, :],
                                    start=dt == 0, stop=dt == DT - 1)
                    return emit

                return [mk(d, d + 4) for d in range(0, DT, 4)], psk

            def kv_v_chunks(tbx):
                """v-projection closures for tbx. Each of the 4 t-chunks gets
                its OWN PSUM tile (one accumulation group per bank — two
                groups in one bank is illegal), sequenced so a chunk is
                copied out to v_all before its bank slot is reused."""
                x_sb = x_tiles[tbx]
                state = {}

                def mk_mm(c, d0, d1):
                    def emit():
                        if d0 == 0:
                            state[c] = ps.tile([P, NKV * H], f32, tag="big",
                                               bufs=6, name=f"psv{c}")
                        for dt in range(d0, d1):
                            nc.tensor.matmul(
                                state[c][:],
                                x_sb[:, dt, c * P:(c + 1) * P],
                                wv_sb[:, dt, :],
                                start=dt == 0, stop=dt == DT - 1)
                    return emit

                def mk_copy(c):
                    def emit():
                        nc.scalar.copy(
                            v_all[:, :, tbx * TC + c, :],
                            state[c][:].rearrange("p (h e) -> p h e", h=NKV))
                    return emit

                chunks = []
                for c in (0, 2):
                    chunks += [mk_mm(c, d, d + 8) for d in range(0, DT, 8)]
                chunks.append(mk_copy(0))
                chunks += [mk_mm(1, d, d + 8) for d in range(0, DT, 8)]
                chunks.append(mk_copy(2))
                chunks += [mk_mm(3, d, d + 8) for d in range(0, DT, 8)]
                chunks.append(mk_copy(1))
                chunks.append(mk_copy(3))
                return chunks, None

            def kv_finish_k(tbx, psk):
                tsl = slice(tbx * TB, (tbx + 1) * TB)
                for kk in range(NKV):
                    rope(kT_all[:, kk, tsl], psk[kk], tbx)

            wqs = {}
            psqs = {}

            def load_wq(h, tb):
                wq_t = wqp.tile([P, DT, H], bf16, tag="wq", name=f"wq{h}")
                for c in range(2):
                    nc.sync.dma_start(
                        wq_t[:, c * 16:(c + 1) * 16, :],
                        wq[c * 16 * P:(c + 1) * 16 * P, h, :]
                        .rearrange("(dt p) e -> p dt e", p=P))
                wqs[h] = wq_t

            def qchunks(h, tb):
                """Closures each emitting a few of head h's 32 accumulating
                q-projection matmuls (wq must already be loading)."""
                x_sb = x_tiles[tb]
                wq_t = wqs[h]
                psq = ps.tile([P, TB], f32, tag="big", bufs=6, name=f"psq{h}")
                psqs[h] = psq

                def mk(d0, d1):
                    def emit():
                        for dt in range(d0, d1):
                            nc.tensor.matmul(psq[:], wq_t[:, dt, :],
                                             x_sb[:, dt, :],
                                             start=dt == 0, stop=dt == DT - 1)
                    return emit

                return [mk(d, min(d + 4, DT)) for d in range(0, DT, 4)]

            def attn(h, tb, qt, enc_h, fills, tail_fn, pe_fills=True,
                     head_fn=None, fin_prev=None):
                """Attention for head h. `fills` are closures interleaved into
                the s-block loop (a later head's q matmuls, or DMA prefetch);
                `tail_fn` (that head's rope) is emitted once fills are done."""
                nsb = TC * (tb + 1)
                ndg = nsb - TC  # non-diagonal s-block count
                kk = h // 4
                enc_ps = ps.tile([P, TB], f32, tag="acc", bufs=2,
                                 name=f"encps{h}")
                exacc = eap.tile([P, TB], bf16, tag="eacc", name=f"eacc{h}")
                look = 5
                ex = [None] * nsb
                tail_state = {"done": tail_fn is None}

                def pre(sb):
                    r = sb - ndg
                    off = P * r if r >= 0 else 0
                    csl = slice(off, TB)
                    lg = ps.tile([P, TB], f32, tag="big", bufs=6,
                                 name=f"lg{sb}")
                    nc.tensor.matmul(lg[:, csl],
                                     kT_all[:, kk, sb * P:(sb + 1) * P],
                                     qt[:, csl])
                    ex_t = ep.tile([P, TB], bf16, tag="ex", name=f"ex{sb}")
                    nc.scalar.activation(ex_t[:, csl], lg[:, csl], Act.Exp,
                                         scale=SCALE)
                    if r >= 0:
                        nc.vector.tensor_tensor(ex_t[:, off:off + P],
                                                ex_t[:, off:off + P],
                                                tri_sb[:], Alu.mult)
                    ex[sb] = (ex_t, csl)

                def post(sb):
                    ex_t, csl = ex[sb]
                    nc.tensor.matmul(enc_ps[:, csl], v_all[:, kk, sb, :],
                                     ex_t[:, csl],
                                     start=sb == 0, stop=sb == nsb - 1)
                    # exp-tile accumulation for the softmax denominator (DVE)
                    if sb == 0:
                        nc.vector.tensor_copy(exacc[:], ex_t[:])
                    else:
                        nc.vector.tensor_tensor(exacc[:, csl], exacc[:, csl],
                                                ex_t[:, csl], Alu.add)

                ci = 0
                while ci < min(2, len(fills)):
                    fills[ci]()
                    ci += 1
                if head_fn is not None:
                    head_fn()
                for sb in range(min(look, nsb)):
                    pre(sb)
                if fin_prev is not None:
                    fin_prev()
                for sb in range(nsb):
                    want = min(len(fills),
                               max((sb + 3) * len(fills) // nsb, 3))
                    while ci < want:
                        fills[ci]()
                        ci += 1
                    if ci == len(fills) and not tail_state["done"]:
                        tail_fn()
                        tail_state["done"] = True
                    post(sb)
                    if sb + look < nsb:
                        pre(sb + look)
                while ci < len(fills):
                    fills[ci]()
                    ci += 1
                if not tail_state["done"]:
                    tail_fn()

                def finalize():
                    lsum = lp.tile([P, TB], f32, tag="lsum", name="lsum")
                    nc.gpsimd.partition_all_reduce(lsum[:], exacc[:], P,
                                                   bass_isa.ReduceOp.add)
                    rinv = lp.tile([P, TB], f32, tag="rinv", name="rinv")
                    nc.vector.reciprocal(rinv[:], lsum[:])
                    nc.vector.tensor_tensor(enc_h[:], enc_ps[:], rinv[:],
                                            Alu.mult)
                return finalize

            wo_tiles = {}

            def load_wo(dc):
                wo_t = wop.tile([P, NQ, TB], bf16, tag="wo", bufs=3,
                                name=f"wo{dc}")
                nc.sync.dma_start(
                    wo_t[:], wo[:, :, dc * TB:(dc + 1) * TB]
                    .rearrange("h p d -> p h d"))
                wo_tiles[dc] = wo_t

            def ph3(tb, enc_hs, nxt, pp=None):
                copy_engs = [nc.scalar.copy, nc.scalar.copy]
                for dc in range(NDC):
                    if nxt is not None and dc == 4:
                        load_wq(2, nxt)
                    if nxt is not None and dc == 6:
                        load_wq(3, nxt)
                    if 2 <= dc + 2 < NDC:
                        load_wo(dc + 2)
                    wo_t = wo_tiles[dc]
                    for tci in range(TC):
                        if pp and dc == 0 and tci in pp:
                            # heads 0..5 pre-accumulated during attn(6)
                            yps = pp[tci]
                            h0 = NQ - 2
                        else:
                            yps = ps.tile([P, TB], f32, tag="big", bufs=6,
                                          name=f"yps{dc}_{tci}")
                            h0 = 0
                        for hh in range(h0, NQ):
                            nc.tensor.matmul(
                                yps[:],
                                enc_hs[hh][:, tci * P:(tci + 1) * P],
                                wo_t[:, hh, :],
                                start=hh == 0, stop=hh == NQ - 1)
                        ys = yp.tile([P, TB], bf16, tag="ys", bufs=3,
                                     name=f"ys{dc}_{tci}")
                        copy_engs[(dc * TC + tci) % 2](ys[:], yps[:])
                        nc.sync.dma_start(
                            y[tb * TB + tci * P:tb * TB + (tci + 1) * P,
                              dc * TB:(dc + 1) * TB], ys[:])

            # ---- startup DMA order: minimum needed for the first k/v
            # matmuls, then the rest ----
            x_tiles = {0: xp.tile([P, DT, TB], bf16, tag="x", name="x0")}
            x_cls0 = x_dma_closures(0, x_tiles[0], tables=False)
            nc.sync.dma_start(wk_sb[:, 0:2], wk_r[:, 0:2])
            nc.sync.dma_start(
                x_tiles[0][:, 0:2, :],
                xT[0:2 * P, 0:TB].rearrange("(g p) t -> p g t", p=P))
            nc.sync.dma_start(wk_sb[:, 2:4], wk_r[:, 2:4])
            nc.sync.dma_start(wv_sb[:, 0:4], wv_r[:, 0:4])
            nc.sync.dma_start(
                x_tiles[0][:, 2:4, :],
                xT[2 * P:4 * P, 0:TB].rearrange("(g p) t -> p g t", p=P))
            x_cls0[1]()
            for ch in range(1, 8):
                c4 = slice(ch * 4, (ch + 1) * 4)
                nc.sync.dma_start(wk_sb[:, c4], wk_r[:, c4])
                nc.sync.dma_start(wv_sb[:, c4], wv_r[:, c4])
                if ch + 1 < 8:
                    x_cls0[ch + 1]()
            nc.sync.dma_start(cos_sb[:, 0:TB], cos[:, 0:TB])
            nc.sync.dma_start(sin_sb[:, 0:TB], sin[:, 0:TB])
            nc.sync.dma_start(tri_sb[:], tri[:])
            load_wq(0, 0)
            load_wq(1, 0)

            # tb0 prologue: k/v projections. k streams per d-tile group as
            # x lands; v granules for chunks 0/2 interleave behind (their
            # d-tile ranges trail the x DMA), the rest follow dense.
            kc0, psk0 = kv_k_chunks(0)
            vcl, _ = kv_v_chunks(0)
            inter = [vcl[0], vcl[4], vcl[1], vcl[5], vcl[2], vcl[6], vcl[3]]
            kc0[0]()
            for g in range(1, 8):
                kc0[g]()
                inter[g - 1]()
            kv_finish_k(0, psk0)
            for b in vcl[7:]:
                b()

            kv_state = {}
            prim_state = {}

            def mk_rope(j, qts, tb):
                qts[j] = qtp.tile([P, TB], bf16, tag="qt", name=f"qt{j}")

                def tail():
                    rope(qts[j], psqs[j], tb)
                return tail

            for tb in range(NTB):
                enc_hs = [encp.tile([P, TB], bf16, tag="ench",
                                    name=f"ench{h}") for h in range(NQ)]
                qts = {}

                # heads 0 and 1: dense q-passes up front (depth-2 priming);
                # rope(0) after both passes, rope(1) deferred into attn(0)
                if tb == 0:
                    load_wq(2, tb)
                for ch in qchunks(0, tb):
                    ch()
                for ch in qchunks(1, tb):
                    ch()
                mk_rope(0, qts, tb)()
                rope1_fn = mk_rope(1, qts, tb)

                nxt = tb + 1 if tb + 1 < NTB else None
                fin = None
                pp = {}

                def pp_mm(tci, ha, hb, enc_hs=enc_hs, pp=pp):
                    def emit():
                        if tci not in pp:
                            pp[tci] = ps.tile([P, TB], f32, tag="big",
                                              bufs=6, name=f"ypsP{tci}")
                        for hh in range(ha, hb):
                            nc.tensor.matmul(
                                pp[tci][:],
                                enc_hs[hh][:, tci * P:(tci + 1) * P],
                                wo_tiles[0][:, hh, :],
                                start=hh == 0, stop=False)
                    return emit

                for h in range(NQ):
                    if h + 3 < NQ and (tb == 0 or h >= 1):
                        load_wq(h + 3, tb)
                    tail_fn = None
                    pe_fills = True
                    if h + 2 < NQ:
                        fills = qchunks(h + 2, tb)
                        tail_fn = mk_rope(h + 2, qts, tb)
                        if nxt is not None and h == 4:
                            # interleave next t-block's x prefetch
                            x_tiles[nxt] = xp.tile([P, DT, TB], bf16,
                                                   tag="x", name=f"x{nxt}")
                            xcl = x_dma_closures(nxt, x_tiles[nxt], nch=4)
                            merged = []
                            for i in range(max(len(fills), len(xcl))):
                                if i < len(fills):
                                    merged.append(fills[i])
                                if i < len(xcl):
                                    merged.append(xcl[i])
                            fills = merged
                        elif nxt is not None and h == 5:
                            fills = fills + [
                                lambda tb=tb: load_wq(0, tb + 1),
                                lambda tb=tb: load_wq(1, tb + 1)]
                    elif h == NQ - 2:
                        # next t-block's k projections + first wo loads
                        fills = [lambda dc=dc: load_wo(dc) for dc in range(2)]
                        if nxt is not None:
                            kc, psk_n = kv_k_chunks(nxt)
                            kv_state["psk"] = psk_n
                            fills = kc + fills
                        else:
                            # last t-block: pre-accumulate heads 0..5 of
                            # ph3's first d-chunk to shorten the tail
                            fills += [pp_mm(0, 0, 3), pp_mm(1, 0, 3),
                                      pp_mm(0, 3, 6), pp_mm(1, 3, 6)]
                    else:
                        # last head: next t-block's v projections, k rope,
                        # v copies
                        fills = []
                        if nxt is not None:
                            vc, _ = kv_v_chunks(nxt)
                            fills = ([vc[0],
                                      lambda: kv_finish_k(nxt,
                                                          kv_state["psk"])] +
                                     vc[1:])
                        else:
                            pe_fills = False
                    fin = attn(h, tb, qts[h], enc_hs[h], fills, tail_fn,
                               pe_fills, head_fn=rope1_fn if h == 0 else None,
                               fin_prev=fin)
                fin()
                ph3(tb, enc_hs, nxt, pp)

    nc.compile()
    return nc


def _get_nc():
    if "nc" not in _STATE:
        _STATE["nc"] = _build_nc()
    return _STATE["nc"]


def _make_in_maps(x, positions, wq, wkv, wo):
    """Build the 8 per-core input dicts (host-side sharding + tables)."""
    B = x.shape[0]
    in_maps = []

    tables = []
    for b in range(B):
        pos = np.asarray(positions[b], np.float64)
        timescale = 10000.0 ** ((2.0 / H) * np.arange(H // 2))
        rad = pos[:, None] / timescale[None, :]          # [T, H/2]
        c64 = np.cos(rad).T                              # [H/2, T]
        s64 = np.sin(rad).T
        tables.append((
            np.ascontiguousarray(np.concatenate([c64, c64], 0)).astype(BF16),
            np.ascontiguousarray(np.concatenate([-s64, s64], 0)).astype(BF16),
        ))

    xTs = [np.ascontiguousarray(x[b].T).astype(BF16) for b in range(B)]

    M = np.zeros((P, P), np.float32)
    for h in range(H // 2):
        M[h, h + H // 2] = -1.0
        M[h + H // 2, h] = 1.0
    mrot = np.ascontiguousarray(M.T).astype(BF16)

    i = np.arange(P)[:, None]
    j = np.arange(P)[None, :]
    tri = np.ascontiguousarray((j >= i).astype(BF16))

    ones = np.ones((P, 1), np.float32)
    onesb = np.ones((P, 1), BF16)

    for c in range(8):
        b, hg = c // 4, c % 4
        qs = slice(NQ * hg, NQ * (hg + 1))
        ks = slice(NKV * hg, NKV * (hg + 1))
        cos_t, sin_t = tables[b]
        in_maps.append({
            "xT": xTs[b],
            "wq": np.ascontiguousarray(wq[qs].transpose(1, 0, 2)).astype(BF16),
            "wk": np.ascontiguousarray(wkv[0, ks].transpose(1, 0, 2)).astype(BF16),
            "wv": np.ascontiguousarray(wkv[1, ks].transpose(1, 0, 2)).astype(BF16),
            "wo": np.ascontiguousarray(wo[qs]).astype(BF16),
            "cos": cos_t,
            "sin": sin_t,
            "mrot": mrot,
            "tri": tri,
            "ones": ones,
            "onesb": onesb,
        })
    return in_maps


def run_cores(in_maps, trace=False, trace_cores=None):
    from concourse.bass_utils import run_bass_kernel_spmd
    nc = _get_nc()
    kw = {}
    if trace:
        kw = dict(trace=True,
                  trace_cores=trace_cores or list(range(8)))
    return run_bass_kernel_spmd(nc, in_maps, core_ids=list(range(8)), **kw)


def kernel(**inputs):
    x = np.asarray(inputs["x"], np.float32)
    positions = np.asarray(inputs["positions"])
    wq = np.asarray(inputs["wq"], np.float32)
    wkv = np.asarray(inputs["wkv"], np.float32)
    wo = np.asarray(inputs["wo"], np.float32)
    B = x.shape[0]
    assert x.shape == (2, T, D) and wq.shape == (32, D, H)

    in_maps = _make_in_maps(x, positions, wq, wkv, wo)
    res = run_cores(in_maps)
    y = np.zeros((B, T, D), np.float32)
    for c, r in enumerate(res.results):
        y[c // 4] += np.asarray(r["y"], np.float32)
    return y


if __name__ == "__main__":
    _build_nc()
    print("build OK")



# revision 12
# speedup vs baseline: 1.0634x; 1.0634x over previous
"""Trainium2 Bass kernel for GQA attention prefill (B=2, T=2048, D=4096, N=32, K=8, H=128).

Sharding: 8 cores = 2 (batch) x 4 (head-groups). Each core handles one batch
element, 8 q-heads and its 2 kv-heads, producing a partial output projection
(summed over its heads). Host sums the 4 partials per batch element (and
undoes the x512 weight scaling).

Precision scheme (PE cost model: bf16/fp16 1.0 cycles/row, fp8+DoubleRow 0.5
cycles/row with a 256-deep contraction -> 4x effective throughput):
  - q/k/v/o projections run as fp8 DoubleRow with hi+lo error compensation:
    w ~ whi + wlo, x ~ xhi + xlo (each e4m3), y = whi@xhi + wlo@xhi + whi@xlo.
    3 quarter-cost matmuls = 0.75x the bf16 cost at ~0.1% error. Weights are
    pre-scaled into e4m3's normal range (wq,wk x64 folded into the exp scale;
    wv x16 cancels against the softmax 1/l fold; wo x32 undone on host).
  - one o-proj head-pair runs direct fp8 (1 matmul, 0.25x cost), spending the
    correctness headroom (~1.7% of final norm).
  - attention (rope, logits, exp, AV) runs in fp16: same PE cost as bf16,
    ~8x lower noise.

Per-core pipeline, software-pipelined per head so PE never idles:
  passA(tb):  k,v projections from xhi/xlo (DMA'd once per t-block, resident
              in SBUF); rope(k) via SBUF->SBUF DMA half-swap plus DVE
              elementwise with fp16 cos/sin tables ([-sin; sin] fold).
  per head h: q-projection matmuls for head h+2 are emitted interleaved with
              head h's attention s-block loop. Attention: logitsT [s128,t<=512]
              = kT-block @ qt (fp16), exp on ACT (scale absorbs the x64 weight
              scales), 0/1 triangle mask multiply on DVE for diagonal tiles,
              AV accumulates in PSUM; denominators accumulate on DVE in f32.
  fin(h):     gpsimd partition reduce -> reciprocal -> DVE psum*rinv -> f32
              tmp, then ACT copy -> enc_hi (fp8) and DVE sub -> enc_lo (fp8),
              pair-interleaved for the o-proj stationary operand.
  ph3(tb):    output projection from enc pair tiles: 3 pairs x 3-term + 1
              direct pair = 10 DoubleRow matmuls per (dchunk, tchunk); PSUM ->
              bf16 SBUF copies on ACT, DMA out per 512-wide d-chunk.
"""

import os
import sys

import numpy as np

for _p in ("/opt/trn_rl_repo", "/root/.axon_site/_ro/trn_rl_repo"):
    if _p not in sys.path and os.path.isdir(_p):
        sys.path.append(_p)

import ml_dtypes

BF16 = ml_dtypes.bfloat16
F16 = np.float16
F8 = ml_dtypes.float8_e4m3fn

P = 128
T = 2048
D = 4096
H = 128
NQ = 8   # q heads per core
NKV = 2  # kv heads per core
TB = 512
NTB = T // TB        # 4
DT = D // P          # 32 d-tiles
NDP = DT // 2        # 16 d-tile pairs
NSB = T // P         # 16 s-blocks
TC = TB // P         # 4 t-chunks per t-block
NDC = D // TB        # 8 d-chunks for the output projection
SCALE = float(H) ** -0.5
SW_QK = 64.0         # wq/wk host scale (folded into exp scale)
SW_V = 16.0          # wv host scale (cancels vs softmax 1/l fold)
SW_O = 32.0          # wo host scale (undone on host with 1/(SW_V*SW_O))
EXP_SCALE = SCALE / (SW_QK * SW_QK)
DIRECT_PAIR = 3      # o-proj head pair computed hi@hi only ...
DIRECT_DCS = frozenset({0, 1, 2, 3})  # ... on these 512-wide d-chunks

_STATE = {}


def _build_nc():
    import concourse.mybir as mybir
    import concourse.tile as tile
    from concourse import bacc
    from concourse import bass_isa

    f32 = mybir.dt.float32
    fp16 = mybir.dt.float16
    fp8 = mybir.dt.float8e4
    bf16 = mybir.dt.bfloat16
    Alu = mybir.AluOpType
    Act = mybir.ActivationFunctionType
    DR = mybir.MatmulPerfMode.DoubleRow

    nc = bacc.Bacc(None, target_bir_lowering=False, debug=False)

    xhi = nc.dram_tensor("xhi", [D, T], fp8, kind="ExternalInput")
    xlo = nc.dram_tensor("xlo", [D, T], fp8, kind="ExternalInput")
    wqh = nc.dram_tensor("wqh", [D, NQ, H], fp8, kind="ExternalInput")
    wql = nc.dram_tensor("wql", [D, NQ, H], fp8, kind="ExternalInput")
    wkh = nc.dram_tensor("wkh", [D, NKV, H], fp8, kind="ExternalInput")
    wkl = nc.dram_tensor("wkl", [D, NKV, H], fp8, kind="ExternalInput")
    wvh = nc.dram_tensor("wvh", [D, NKV, H], fp8, kind="ExternalInput")
    wvl = nc.dram_tensor("wvl", [D, NKV, H], fp8, kind="ExternalInput")
    # o-projection weights, head-major within rows: [H, NQ, D] (adjacent
    # heads form the DoubleRow pairs)
    woh = nc.dram_tensor("woh", [H, NQ, D], fp8, kind="ExternalInput")
    wol = nc.dram_tensor("wol", [H, NQ, D], fp8, kind="ExternalInput")
    cos = nc.dram_tensor("cos", [P, T], fp16, kind="ExternalInput")
    sin = nc.dram_tensor("sin", [P, T], fp16, kind="ExternalInput")
    tri = nc.dram_tensor("tri", [P, P], fp16, kind="ExternalInput")
    y = nc.dram_tensor("y", [T, D], bf16, kind="ExternalOutput")

    with tile.TileContext(nc) as tc:
        with (
            tc.tile_pool(name="const", bufs=1) as const,
            tc.tile_pool(name="xp", bufs=2) as xp,
            tc.tile_pool(name="wqp", bufs=2) as wqp,
            tc.tile_pool(name="qtp", bufs=3) as qtp,
            tc.tile_pool(name="rp", bufs=3) as rp,
            tc.tile_pool(name="ep", bufs=6) as ep,
            tc.tile_pool(name="eap", bufs=2) as eap,
            tc.tile_pool(name="encp", bufs=2) as encp,
            tc.tile_pool(name="lp", bufs=1) as lp,
            tc.tile_pool(name="wop", bufs=2) as wop,
            tc.tile_pool(name="yp", bufs=2) as yp,
            tc.tile_pool(name="ps", bufs=1, space="PSUM") as ps,
        ):
            wk_hi = const.tile([P, DT, NKV, H], fp8, tag="wkh")
            wk_lo = const.tile([P, DT, NKV, H], fp8, tag="wkl")
            wv_hi = const.tile([P, DT, NKV * H], fp8, tag="wvh")
            wv_lo = const.tile([P, DT, NKV * H], fp8, tag="wvl")
            wkh_r = wkh.rearrange("(dt p) h e -> p dt h e", p=P)
            wkl_r = wkl.rearrange("(dt p) h e -> p dt h e", p=P)
            wvh_r = wvh.rearrange("(dt p) h e -> p dt (h e)", p=P)
            wvl_r = wvl.rearrange("(dt p) h e -> p dt (h e)", p=P)
            tri_sb = const.tile([P, P], fp16, tag="tri")
            cos_sb = const.tile([P, T], fp16, tag="cos")
            sin_sb = const.tile([P, T], fp16, tag="sin")
            kT_all = const.tile([P, NKV, T], fp16, tag="kT")
            v_all = const.tile([P, NKV, NSB, H], fp16, tag="v")

            def x_dma_closures(tb, x_hi, x_lo, tables=True, nch=8):
                """nch closures, each DMA-ing a DT/nch-d-tile chunk of
                xhi+xlo for tb (plus this t-block's cos/sin on chunk 0)."""
                tsl = slice(tb * TB, (tb + 1) * TB)
                step = DT // nch

                def mk(c8):
                    def emit():
                        dsl = slice(c8 * step * P, (c8 + 1) * step * P)
                        csl = slice(c8 * step, (c8 + 1) * step)
                        nc.sync.dma_start(
                            x_hi[:, csl, :],
                            xhi[dsl, tsl].rearrange("(g p) t -> p g t", p=P))
                        nc.sync.dma_start(
                            x_lo[:, csl, :],
                            xlo[dsl, tsl].rearrange("(g p) t -> p g t", p=P))
                        if c8 == 0 and tables:
                            nc.sync.dma_start(cos_sb[:, tsl], cos[:, tsl])
                            nc.sync.dma_start(sin_sb[:, tsl], sin[:, tsl])
                    return emit

                return [mk(c8) for c8 in range(nch)]

            def mm3(out, w_hi, w_lo, m_hi, m_lo, pr, start, stop):
                """3-term compensated DoubleRow accumulation for d-pair pr.
                w_* slices must be [P, 2, M] fp8, m_* [P, 2, N] fp8."""
                sl = slice(2 * pr, 2 * pr + 2)
                nc.tensor.matmul(out, w_hi[:, sl, :], m_hi[:, sl, :],
                                 start=start, stop=False, perf_mode=DR)
                nc.tensor.matmul(out, w_lo[:, sl, :], m_hi[:, sl, :],
                                 start=False, stop=False, perf_mode=DR)
                nc.tensor.matmul(out, w_hi[:, sl, :], m_lo[:, sl, :],
                                 start=False, stop=stop, perf_mode=DR)

            def rope(dst, src_ps, tb):
                """dst[:] = rope(src_ps) for one head's [H, TB] block (fp16).
                Half-swap via SBUF->SBUF DMA partition reorder; the sign of
                the swapped half is folded into the sin table ([-sin; +sin])."""
                cs = cos_sb[:, tb * TB:(tb + 1) * TB]
                sn = sin_sb[:, tb * TB:(tb + 1) * TB]
                raw = rp.tile([P, TB], fp16, tag="raw", name="raw")
                nc.scalar.copy(raw[:], src_ps[:])
                shuf = rp.tile([P, TB], fp16, tag="shuf", name="shuf")
                nc.sync.dma_start(shuf[0:P // 2, :], raw[P // 2:P, :])
                nc.sync.dma_start(shuf[P // 2:P, :], raw[0:P // 2, :])
                tmp = rp.tile([P, TB], fp16, tag="tmp", name="tmp")
                nc.vector.tensor_tensor(dst, raw[:], cs, Alu.mult)
                nc.vector.tensor_tensor(tmp[:], shuf[:], sn, Alu.mult)
                nc.vector.tensor_tensor(dst, dst, tmp[:], Alu.add)

            def kv_k_chunks(tbx):
                """k-projection chunk closures (+ psk tiles) for tbx."""
                x_hi, x_lo = x_tiles[tbx]
                psk = [ps.tile([P, TB], f32, tag="big", bufs=6,
                               name=f"psk{_k}") for _k in range(NKV)]

                def mk(p0, p1):
                    def emit():
                        for pr in range(p0, p1):
                            for kk in range(NKV):
                                mm3(psk[kk][:],
                                    wk_hi[:, :, kk, :], wk_lo[:, :, kk, :],
                                    x_hi, x_lo, pr,
                                    start=pr == 0, stop=pr == NDP - 1)
                    return emit

                return [mk(p, p + 2) for p in range(0, NDP, 2)], psk

            def kv_v_chunks(tbx):
                """v-projection closures for tbx. Each of the 4 t-chunks gets
                its OWN PSUM tile, sequenced so a chunk is copied out to
                v_all before its bank slot is reused."""
                x_hi, x_lo = x_tiles[tbx]
                state = {}

                def mk_mm(c, p0, p1):
                    def emit():
                        if p0 == 0:
                            state[c] = ps.tile([P, NKV * H], f32, tag="big",
                                               bufs=6, name=f"psv{c}")
                        csl = slice(c * P, (c + 1) * P)
                        for pr in range(p0, p1):
                            mm3(state[c][:],
                                x_hi[:, :, csl], x_lo[:, :, csl],
                                wv_hi, wv_lo, pr,
                                start=pr == 0, stop=pr == NDP - 1)
                    return emit

                def mk_copy(c):
                    def emit():
                        nc.scalar.copy(
                            v_all[:, :, tbx * TC + c, :],
                            state[c][:].rearrange("p (h e) -> p h e", h=NKV))
                    return emit

                chunks = []
                for c in (0, 2):
                    chunks += [mk_mm(c, p, p + 4) for p in range(0, NDP, 4)]
                chunks.append(mk_copy(0))
                chunks += [mk_mm(1, p, p + 4) for p in range(0, NDP, 4)]
                chunks.append(mk_copy(2))
                chunks += [mk_mm(3, p, p + 4) for p in range(0, NDP, 4)]
                chunks.append(mk_copy(1))
                chunks.append(mk_copy(3))
                return chunks, None

            def kv_finish_k(tbx, psk):
                tsl = slice(tbx * TB, (tbx + 1) * TB)
                for kk in range(NKV):
                    rope(kT_all[:, kk, tsl], psk[kk], tbx)

            wqs = {}
            psqs = {}

            def load_wq(h, tb):
                wq_hi = wqp.tile([P, DT, H], fp8, tag="wqh", name=f"wqh{h}")
                wq_lo = wqp.tile([P, DT, H], fp8, tag="wql", name=f"wql{h}")
                for c in range(2):
                    dsl = slice(c * 16 * P, (c + 1) * 16 * P)
                    csl = slice(c * 16, (c + 1) * 16)
                    nc.sync.dma_start(
                        wq_hi[:, csl, :],
                        wqh[dsl, h, :].rearrange("(dt p) e -> p dt e", p=P))
                    nc.sync.dma_start(
                        wq_lo[:, csl, :],
                        wql[dsl, h, :].rearrange("(dt p) e -> p dt e", p=P))
                wqs[h] = (wq_hi, wq_lo)

            def qchunks(h, tb):
                """Closures each emitting a few of head h's 48 accumulating
                q-projection DoubleRow matmuls (wq must already be loading)."""
                x_hi, x_lo = x_tiles[tb]
                wq_hi, wq_lo = wqs[h]
                psq = ps.tile([P, TB], f32, tag="big", bufs=6, name=f"psq{h}")
                psqs[h] = psq

                def mk(p0, p1):
                    def emit():
                        for pr in range(p0, p1):
                            mm3(psq[:], wq_hi, wq_lo, x_hi, x_lo, pr,
                                start=pr == 0, stop=pr == NDP - 1)
                    return emit

                return [mk(p, p + 2) for p in range(0, NDP, 2)]

            def attn(h, tb, qt, hp, fills, tail_fn, pe_fills=True,
                     head_fn=None, fin_prev=None):
                """Attention for head h. `fills` are closures interleaved into
                the s-block loop (a later head's q matmuls, or DMA prefetch);
                `tail_fn` (that head's rope) is emitted once fills are done."""
                nsb = TC * (tb + 1)
                ndg = nsb - TC  # non-diagonal s-block count
                kk = h // 4
                enc_ps = ps.tile([P, TB], f32, tag="acc", bufs=2,
                                 name=f"encps{h}")
                exacc = eap.tile([P, TB], f32, tag="eacc", name=f"eacc{h}")
                look = 5
                ex = [None] * nsb
                tail_state = {"done": tail_fn is None}

                def pre(sb):
                    r = sb - ndg
                    off = P * r if r >= 0 else 0
                    csl = slice(off, TB)
                    lg = ps.tile([P, TB], f32, tag="big", bufs=6,
                                 name=f"lg{sb}")
                    nc.tensor.matmul(lg[:, csl],
                                     kT_all[:, kk, sb * P:(sb + 1) * P],
                                     qt[:, csl])
                    ex_t = ep.tile([P, TB], fp16, tag="ex", name=f"ex{sb}")
                    nc.scalar.activation(ex_t[:, csl], lg[:, csl], Act.Exp,
                                         scale=EXP_SCALE)
                    if r >= 0:
                        nc.vector.tensor_tensor(ex_t[:, off:off + P],
                                                ex_t[:, off:off + P],
                                                tri_sb[:], Alu.mult)
                    ex[sb] = (ex_t, csl)

                def post(sb):
                    ex_t, csl = ex[sb]
                    nc.tensor.matmul(enc_ps[:, csl], v_all[:, kk, sb, :],
                                     ex_t[:, csl],
                                     start=sb == 0, stop=sb == nsb - 1)
                    # exp-tile accumulation for the softmax denominator (DVE)
                    if sb == 0:
                        nc.vector.tensor_copy(exacc[:], ex_t[:])
                    else:
                        nc.vector.tensor_tensor(exacc[:, csl], exacc[:, csl],
                                                ex_t[:, csl], Alu.add)

                ci = 0
                while ci < min(2, len(fills)):
                    fills[ci]()
                    ci += 1
                if head_fn is not None:
                    head_fn()
                for sb in range(min(look, nsb)):
                    pre(sb)
                if fin_prev is not None:
                    fin_prev()
                for sb in range(nsb):
                    want = min(len(fills),
                               max((sb + 3) * len(fills) // nsb, 3))
                    while ci < want:
                        fills[ci]()
                        ci += 1
                    if ci == len(fills) and not tail_state["done"]:
                        tail_fn()
                        tail_state["done"] = True
                    post(sb)
                    if sb + look < nsb:
                        pre(sb + look)
                while ci < len(fills):
                    fills[ci]()
                    ci += 1
                if not tail_state["done"]:
                    tail_fn()

                def finalize():
                    lsum = lp.tile([P, TB], f32, tag="lsum", name="lsum")
                    nc.gpsimd.partition_all_reduce(lsum[:], exacc[:], P,
                                                   bass_isa.ReduceOp.add)
                    rinv = lp.tile([P, TB], f32, tag="rinv", name="rinv")
                    nc.vector.reciprocal(rinv[:], lsum[:])
                    tmp = lp.tile([P, TB], f32, tag="etmp", name="etmp")
                    nc.vector.tensor_tensor(tmp[:], enc_ps[:], rinv[:],
                                            Alu.mult)
                    pr, parity = h // 2, h % 2
                    hi_sl = enc_hi[pr][:, parity, :]
                    nc.scalar.copy(hi_sl, tmp[:])
                    nc.vector.tensor_tensor(enc_lo[pr][:, parity, :],
                                            tmp[:], hi_sl, Alu.subtract)
                return finalize

            wo_tiles = {}

            def load_wo(dc):
                dsl = slice(dc * TB, (dc + 1) * TB)
                wo_h = wop.tile([P, NQ, TB], fp8, tag="woh", bufs=3,
                                name=f"woh{dc}")
                nc.sync.dma_start(wo_h[:], woh[:, :, dsl])
                wo_l = wop.tile([P, NQ, TB], fp8, tag="wol", bufs=3,
                                name=f"wol{dc}")
                nc.sync.dma_start(wo_l[:], wol[:, :, dsl])
                wo_tiles[dc] = (wo_h, wo_l)

            def oproj_pair(yps, pr, dc, tci, start, stop):
                """o-projection matmuls for head pair pr into yps.
                3-term compensated, or single hi@hi for the direct pair on
                the direct d-chunks."""
                wo_h, wo_l = wo_tiles[dc]
                tsl = slice(tci * P, (tci + 1) * P)
                psl = slice(2 * pr, 2 * pr + 2)
                e_hi = enc_hi[pr][:, :, tsl]
                direct = pr == DIRECT_PAIR and dc in DIRECT_DCS
                nc.tensor.matmul(yps, e_hi, wo_h[:, psl, :], start=start,
                                 stop=stop and direct, perf_mode=DR)
                if not direct:
                    nc.tensor.matmul(yps, e_hi, wo_l[:, psl, :], start=False,
                                     stop=False, perf_mode=DR)
                    nc.tensor.matmul(yps, enc_lo[pr][:, :, tsl],
                                     wo_h[:, psl, :], start=False, stop=stop,
                                     perf_mode=DR)

            def ph3(tb, nxt, pp=None):
                for dc in range(NDC):
                    if nxt is not None and dc == 4:
                        load_wq(2, nxt)
                    if nxt is not None and dc == 6:
                        load_wq(3, nxt)
                    if 2 <= dc + 2 < NDC:
                        load_wo(dc + 2)
                    for tci in range(TC):
                        if pp and dc == 0 and tci in pp:
                            # pairs 0..2 pre-accumulated during attn(6)
                            yps = pp[tci]
                            prs = [DIRECT_PAIR]
                        else:
                            yps = ps.tile([P, TB], f32, tag="big", bufs=6,
                                          name=f"yps{dc}_{tci}")
                            prs = list(range(NQ // 2))
                        for i, pr in enumerate(prs):
                            oproj_pair(yps[:], pr, dc, tci,
                                       start=i == 0 and len(prs) == 4,
                                       stop=i == len(prs) - 1)
                        ys = yp.tile([P, TB], bf16, tag="ys", bufs=3,
                                     name=f"ys{dc}_{tci}")
                        nc.scalar.copy(ys[:], yps[:])
                        nc.sync.dma_start(
                            y[tb * TB + tci * P:tb * TB + (tci + 1) * P,
                              dc * TB:(dc + 1) * TB], ys[:])

            # ---- startup DMA order: minimum needed for the first k/v
            # matmuls, then the rest ----
            x_tiles = {0: (xp.tile([P, DT, TB], fp8, tag="xh", name="xh0"),
                           xp.tile([P, DT, TB], fp8, tag="xl", name="xl0"))}
            x_cls0 = x_dma_closures(0, *x_tiles[0], tables=False)
            xh0, xl0 = x_tiles[0]
            nc.sync.dma_start(wk_hi[:, 0:2], wkh_r[:, 0:2])
            nc.sync.dma_start(wk_lo[:, 0:2], wkl_r[:, 0:2])
            nc.sync.dma_start(
                xh0[:, 0:2, :],
                xhi[0:2 * P, 0:TB].rearrange("(g p) t -> p g t", p=P))
            nc.sync.dma_start(
                xl0[:, 0:2, :],
                xlo[0:2 * P, 0:TB].rearrange("(g p) t -> p g t", p=P))
            nc.sync.dma_start(wk_hi[:, 2:4], wkh_r[:, 2:4])
            nc.sync.dma_start(wk_lo[:, 2:4], wkl_r[:, 2:4])
            nc.sync.dma_start(wv_hi[:, 0:4], wvh_r[:, 0:4])
            nc.sync.dma_start(wv_lo[:, 0:4], wvl_r[:, 0:4])
            nc.sync.dma_start(
                xh0[:, 2:4, :],
                xhi[2 * P:4 * P, 0:TB].rearrange("(g p) t -> p g t", p=P))
            nc.sync.dma_start(
                xl0[:, 2:4, :],
                xlo[2 * P:4 * P, 0:TB].rearrange("(g p) t -> p g t", p=P))
            x_cls0[1]()
            for ch in range(1, 8):
                c4 = slice(ch * 4, (ch + 1) * 4)
                nc.sync.dma_start(wk_hi[:, c4], wkh_r[:, c4])
                nc.sync.dma_start(wk_lo[:, c4], wkl_r[:, c4])
                nc.sync.dma_start(wv_hi[:, c4], wvh_r[:, c4])
                nc.sync.dma_start(wv_lo[:, c4], wvl_r[:, c4])
                if ch + 1 < 8:
                    x_cls0[ch + 1]()
            nc.sync.dma_start(cos_sb[:, 0:TB], cos[:, 0:TB])
            nc.sync.dma_start(sin_sb[:, 0:TB], sin[:, 0:TB])
            nc.sync.dma_start(tri_sb[:], tri[:])
            load_wq(0, 0)
            load_wq(1, 0)

            # tb0 prologue: k/v projections. k streams per d-pair group as
            # x lands; v granules interleave behind.
            kc0, psk0 = kv_k_chunks(0)
            vcl, _ = kv_v_chunks(0)
            inter = [vcl[0], vcl[4], vcl[1], vcl[5], vcl[2], vcl[6], vcl[3]]
            kc0[0]()
            for g in range(1, 8):
                kc0[g]()
                inter[g - 1]()
            kv_finish_k(0, psk0)
            for b in vcl[7:]:
                b()

            kv_state = {}

            def mk_rope(j, qts, tb):
                qts[j] = qtp.tile([P, TB], fp16, tag="qt", name=f"qt{j}")

                def tail():
                    rope(qts[j], psqs[j], tb)
                return tail

            for tb in range(NTB):
                enc_hi = [encp.tile([P, 2, TB], fp8, tag=f"ehi{pr}",
                                    name=f"ehi{pr}") for pr in range(4)]
                enc_lo = [encp.tile([P, 2, TB], fp8, tag=f"elo{pr}",
                                    name=f"elo{pr}") for pr in range(4)]
                qts = {}

                # heads 0 and 1: dense q-passes up front (depth-2 priming);
                # rope(0) after both passes, rope(1) deferred into attn(0)
                if tb == 0:
                    load_wq(2, tb)
                for ch in qchunks(0, tb):
                    ch()
                for ch in qchunks(1, tb):
                    ch()
                mk_rope(0, qts, tb)()
                rope1_fn = mk_rope(1, qts, tb)

                nxt = tb + 1 if tb + 1 < NTB else None
                fin = None
                pp = {}

                def pp_mm(tci, pa, pb, pp=pp):
                    def emit():
                        if tci not in pp:
                            pp[tci] = ps.tile([P, TB], f32, tag="big",
                                              bufs=6, name=f"ypsP{tci}")
                        for pr in range(pa, pb):
                            oproj_pair(pp[tci][:], pr, 0, tci,
                                       start=pr == 0, stop=False)
                    return emit

                for h in range(NQ):
                    if h + 3 < NQ and (tb == 0 or h >= 1):
                        load_wq(h + 3, tb)
                    tail_fn = None
                    pe_fills = True
                    if h + 2 < NQ:
                        fills = qchunks(h + 2, tb)
                        tail_fn = mk_rope(h + 2, qts, tb)
                        if nxt is not None and h == 4:
                            # interleave next t-block's x prefetch
                            x_tiles[nxt] = (
                                xp.tile([P, DT, TB], fp8, tag="xh",
                                        name=f"xh{nxt}"),
                                xp.tile([P, DT, TB], fp8, tag="xl",
                                        name=f"xl{nxt}"))
                            xcl = x_dma_closures(nxt, *x_tiles[nxt], nch=4)
                            merged = []
                            for i in range(max(len(fills), len(xcl))):
                                if i < len(fills):
                                    merged.append(fills[i])
                                if i < len(xcl):
                                    merged.append(xcl[i])
                            fills = merged
                        elif nxt is not None and h == 5:
                            fills = fills + [
                                lambda tb=tb: load_wq(0, tb + 1),
                                lambda tb=tb: load_wq(1, tb + 1)]
                    elif h == NQ - 2:
                        # next t-block's k projections + first wo loads
                        fills = [lambda dc=dc: load_wo(dc) for dc in range(2)]
                        if nxt is not None:
                            kc, psk_n = kv_k_chunks(nxt)
                            kv_state["psk"] = psk_n
                            fills = kc + fills
                        else:
                            # last t-block: pre-accumulate pairs 0..2 of
                            # ph3's first d-chunk to shorten the tail
                            fills += [pp_mm(0, 0, 2), pp_mm(1, 0, 2),
                                      pp_mm(0, 2, 3), pp_mm(1, 2, 3)]
                    else:
                        # last head: next t-block's v projections, k rope,
                        # v copies
                        fills = []
                        if nxt is not None:
                            vc, _ = kv_v_chunks(nxt)
                            fills = ([vc[0],
                                      lambda: kv_finish_k(nxt,
                                                          kv_state["psk"])] +
                                     vc[1:])
                        else:
                            pe_fills = False
                    fin = attn(h, tb, qts[h], h // 2, fills, tail_fn,
                               pe_fills, head_fn=rope1_fn if h == 0 else None,
                               fin_prev=fin)
                fin()
                ph3(tb, nxt, pp)

    nc.compile()
    return nc


def _get_nc():
    if "nc" not in _STATE:
        _STATE["nc"] = _build_nc()
    return _STATE["nc"]


def _q8(a):
    return np.ascontiguousarray(a, dtype=np.float32).astype(F8)


def _hilo(a):
    hi = _q8(a)
    lo = _q8(np.asarray(a, np.float32) - hi.astype(np.float32))
    return hi, lo


def _make_in_maps(x, positions, wq, wkv, wo):
    """Build the 8 per-core input dicts (host-side quantization + tables)."""
    B = x.shape[0]
    in_maps = []

    tables = []
    for b in range(B):
        pos = np.asarray(positions[b], np.float64)
        timescale = 10000.0 ** ((2.0 / H) * np.arange(H // 2))
        rad = pos[:, None] / timescale[None, :]          # [T, H/2]
        c64 = np.cos(rad).T                              # [H/2, T]
        s64 = np.sin(rad).T
        tables.append((
            np.ascontiguousarray(np.concatenate([c64, c64], 0)).astype(F16),
            np.ascontiguousarray(np.concatenate([-s64, s64], 0)).astype(F16),
        ))

    xThilo = [_hilo(np.ascontiguousarray(x[b].T)) for b in range(B)]

    i = np.arange(P)[:, None]
    j = np.arange(P)[None, :]
    tri = np.ascontiguousarray((j >= i).astype(F16))

    for c in range(8):
        b, hg = c // 4, c % 4
        qs = slice(NQ * hg, NQ * (hg + 1))
        ks = slice(NKV * hg, NKV * (hg + 1))
        cos_t, sin_t = tables[b]
        xh, xl = xThilo[b]
        wq_h, wq_l = _hilo(SW_QK * wq[qs].transpose(1, 0, 2))
        wk_h, wk_l = _hilo(SW_QK * wkv[0, ks].transpose(1, 0, 2))
        wv_h, wv_l = _hilo(SW_V * wkv[1, ks].transpose(1, 0, 2))
        # wo: [8, H, D] -> [H, 8, D]; adjacent heads form DoubleRow pairs
        wo_h, wo_l = _hilo((SW_O * wo[qs]).transpose(1, 0, 2))
        in_maps.append({
            "xhi": xh,
            "xlo": xl,
            "wqh": wq_h,
            "wql": wq_l,
            "wkh": wk_h,
            "wkl": wk_l,
            "wvh": wv_h,
            "wvl": wv_l,
            "woh": wo_h,
            "wol": wo_l,
            "cos": cos_t,
            "sin": sin_t,
            "tri": tri,
        })
    return in_maps


def run_cores(in_maps, trace=False, trace_cores=None):
    from concourse.bass_utils import run_bass_kernel_spmd
    nc = _get_nc()
    kw = {}
    if trace:
        kw = dict(trace=True,
                  trace_cores=trace_cores or list(range(8)))
    return run_bass_kernel_spmd(nc, in_maps, core_ids=list(range(8)), **kw)


def kernel(**inputs):
    x = np.asarray(inputs["x"], np.float32)
    positions = np.asarray(inputs["positions"])
    wq = np.asarray(inputs["wq"], np.float32)
    wkv = np.asarray(inputs["wkv"], np.float32)
    wo = np.asarray(inputs["wo"], np.float32)
    B = x.shape[0]
    assert x.shape == (2, T, D) and wq.shape == (32, D, H)

    in_maps = _make_in_maps(x, positions, wq, wkv, wo)
    res = run_cores(in_maps)
    y = np.zeros((B, T, D), np.float32)
    inv = 1.0 / (SW_V * SW_O)
    for c, r in enumerate(res.results):
        y[c // 4] += np.asarray(r["y"], np.float32) * inv
    return y


if __name__ == "__main__":
    _build_nc()
    print("build OK")


# revision 16
# speedup vs baseline: 1.0986x; 1.0331x over previous
"""Trainium2 Bass kernel for GQA attention prefill (B=2, T=2048, D=4096, N=32, K=8, H=128).

Sharding: 8 cores = 2 (batch) x 4 (head-groups). Each core handles one batch
element, 8 q-heads and its 2 kv-heads, producing a partial output projection
(summed over its heads). Host sums the 4 partials per batch element (and
undoes the x512 weight scaling).

Precision scheme (PE cost model: bf16/fp16 1.0 cycles/row, fp8+DoubleRow 0.5
cycles/row with a 256-deep contraction -> 4x effective throughput):
  - q/k/v/o projections run as fp8 DoubleRow with hi+lo error compensation:
    w ~ whi + wlo, x ~ xhi + xlo (each e4m3), y = whi@xhi + wlo@xhi + whi@xlo.
    3 quarter-cost matmuls = 0.75x the bf16 cost at ~0.1% error. Weights are
    pre-scaled into e4m3's normal range (wq,wk x64 folded into the exp scale;
    wv x16 cancels against the softmax 1/l fold; wo x32 undone on host).
  - one o-proj head-pair runs direct fp8 (1 matmul, 0.25x cost), spending the
    correctness headroom (~1.7% of final norm).
  - attention (rope, logits, exp, AV) runs in fp16: same PE cost as bf16,
    ~8x lower noise.

Per-core pipeline, software-pipelined per head so PE never idles:
  passA(tb):  k,v projections from xhi/xlo (DMA'd once per t-block, resident
              in SBUF); rope(k) via SBUF->SBUF DMA half-swap plus DVE
              elementwise with fp16 cos/sin tables ([-sin; sin] fold).
  per head h: q-projection matmuls for head h+2 are emitted interleaved with
              head h's attention s-block loop. Attention: logitsT [s128,t<=512]
              = kT-block @ qt (fp16), exp on ACT (scale absorbs the x64 weight
              scales), 0/1 triangle mask multiply on DVE for diagonal tiles,
              AV accumulates in PSUM; denominators accumulate on DVE in f32.
  fin(h):     gpsimd partition reduce -> reciprocal -> DVE psum*rinv -> f32
              tmp, then ACT copy -> enc_hi (fp8) and DVE sub -> enc_lo (fp8),
              pair-interleaved for the o-proj stationary operand.
  ph3(tb):    output projection from enc pair tiles: 3 pairs x 3-term + 1
              direct pair = 10 DoubleRow matmuls per (dchunk, tchunk); PSUM ->
              bf16 SBUF copies on ACT, DMA out per 512-wide d-chunk.
"""

import os
import sys

import numpy as np

for _p in ("/opt/trn_rl_repo", "/root/.axon_site/_ro/trn_rl_repo"):
    if _p not in sys.path and os.path.isdir(_p):
        sys.path.append(_p)

import ml_dtypes

BF16 = ml_dtypes.bfloat16
F16 = np.float16
F8 = ml_dtypes.float8_e4m3fn

P = 128
T = 2048
D = 4096
H = 128
NQ = 8   # q heads per core
NKV = 2  # kv heads per core
TB = 512
NTB = T // TB        # 4
DT = D // P          # 32 d-tiles
NDP = DT // 2        # 16 d-tile pairs
NSB = T // P         # 16 s-blocks
TC = TB // P         # 4 t-chunks per t-block
NDC = D // TB        # 8 d-chunks for the output projection
SCALE = float(H) ** -0.5
SW_QK = 64.0         # wq/wk host scale (folded into exp scale)
SW_V = 16.0          # wv host scale (cancels vs softmax 1/l fold)
SW_O = 32.0          # wo host scale (undone on host with 1/(SW_V*SW_O))
EXP_SCALE = SCALE / (SW_QK * SW_QK)
DIRECT_PAIR = 3      # o-proj head pair computed hi@hi only ...
DIRECT_DCS = frozenset({0, 1, 2, 3})  # ... on these 512-wide d-chunks

_STATE = {}


def _build_nc():
    import concourse.mybir as mybir
    import concourse.tile as tile
    from concourse import bacc
    from concourse import bass_isa

    f32 = mybir.dt.float32
    fp16 = mybir.dt.float16
    fp8 = mybir.dt.float8e4
    bf16 = mybir.dt.bfloat16
    Alu = mybir.AluOpType
    Act = mybir.ActivationFunctionType
    DR = mybir.MatmulPerfMode.DoubleRow

    nc = bacc.Bacc(None, target_bir_lowering=False, debug=False)

    xhi = nc.dram_tensor("xhi", [D, T], fp8, kind="ExternalInput")
    xlo = nc.dram_tensor("xlo", [D, T], fp8, kind="ExternalInput")
    wqh = nc.dram_tensor("wqh", [D, NQ, H], fp8, kind="ExternalInput")
    wql = nc.dram_tensor("wql", [D, NQ, H], fp8, kind="ExternalInput")
    wkh = nc.dram_tensor("wkh", [D, NKV, H], fp8, kind="ExternalInput")
    wkl = nc.dram_tensor("wkl", [D, NKV, H], fp8, kind="ExternalInput")
    wvh = nc.dram_tensor("wvh", [D, NKV, H], fp8, kind="ExternalInput")
    wvl = nc.dram_tensor("wvl", [D, NKV, H], fp8, kind="ExternalInput")
    # o-projection weights, head-major within rows: [H, NQ, D] (adjacent
    # heads form the DoubleRow pairs)
    woh = nc.dram_tensor("woh", [H, NQ, D], fp8, kind="ExternalInput")
    wol = nc.dram_tensor("wol", [H, NQ, D], fp8, kind="ExternalInput")
    cos = nc.dram_tensor("cos", [P, T], fp16, kind="ExternalInput")
    sin = nc.dram_tensor("sin", [P, T], fp16, kind="ExternalInput")
    tri = nc.dram_tensor("tri", [P, P], fp16, kind="ExternalInput")
    y = nc.dram_tensor("y", [T, D], bf16, kind="ExternalOutput")

    with tile.TileContext(nc) as tc:
        with (
            tc.tile_pool(name="const", bufs=1) as const,
            tc.tile_pool(name="xp", bufs=2) as xp,
            tc.tile_pool(name="wqp", bufs=2) as wqp,
            tc.tile_pool(name="qtp", bufs=3) as qtp,
            tc.tile_pool(name="rp", bufs=3) as rp,
            tc.tile_pool(name="ep", bufs=6) as ep,
            tc.tile_pool(name="eap", bufs=2) as eap,
            tc.tile_pool(name="encp", bufs=2) as encp,
            tc.tile_pool(name="lp", bufs=1) as lp,
            tc.tile_pool(name="wop", bufs=2) as wop,
            tc.tile_pool(name="yp", bufs=2) as yp,
            tc.tile_pool(name="ps", bufs=1, space="PSUM") as ps,
        ):
            wk_hi = const.tile([P, DT, NKV, H], fp8, tag="wkh")
            wk_lo = const.tile([P, DT, NKV, H], fp8, tag="wkl")
            wv_hi = const.tile([P, DT, NKV * H], fp8, tag="wvh")
            wv_lo = const.tile([P, DT, NKV * H], fp8, tag="wvl")
            wkh_r = wkh.rearrange("(dt p) h e -> p dt h e", p=P)
            wkl_r = wkl.rearrange("(dt p) h e -> p dt h e", p=P)
            wvh_r = wvh.rearrange("(dt p) h e -> p dt (h e)", p=P)
            wvl_r = wvl.rearrange("(dt p) h e -> p dt (h e)", p=P)
            tri_sb = const.tile([P, P], fp16, tag="tri")
            cos_sb = const.tile([P, T], fp16, tag="cos")
            sin_sb = const.tile([P, T], fp16, tag="sin")
            kT_all = const.tile([P, NKV, T], fp16, tag="kT")
            v_all = const.tile([P, NKV, NSB, H], fp16, tag="v")

            def x_dma_closures(tb, x_hi, x_lo, tables=True, nch=8):
                """nch closures, each DMA-ing a DT/nch-d-tile chunk of
                xhi+xlo for tb (plus this t-block's cos/sin on chunk 0)."""
                tsl = slice(tb * TB, (tb + 1) * TB)
                step = DT // nch

                def mk(c8):
                    def emit():
                        dsl = slice(c8 * step * P, (c8 + 1) * step * P)
                        csl = slice(c8 * step, (c8 + 1) * step)
                        nc.sync.dma_start(
                            x_hi[:, csl, :],
                            xhi[dsl, tsl].rearrange("(g p) t -> p g t", p=P))
                        nc.sync.dma_start(
                            x_lo[:, csl, :],
                            xlo[dsl, tsl].rearrange("(g p) t -> p g t", p=P))
                        if c8 == 0 and tables:
                            nc.sync.dma_start(cos_sb[:, tsl], cos[:, tsl])
                            nc.sync.dma_start(sin_sb[:, tsl], sin[:, tsl])
                    return emit

                return [mk(c8) for c8 in range(nch)]

            def mm3(out, w_hi, w_lo, m_hi, m_lo, pr, start, stop):
                """3-term compensated DoubleRow accumulation for d-pair pr.
                w_* slices must be [P, 2, M] fp8, m_* [P, 2, N] fp8."""
                sl = slice(2 * pr, 2 * pr + 2)
                nc.tensor.matmul(out, w_hi[:, sl, :], m_hi[:, sl, :],
                                 start=start, stop=False, perf_mode=DR)
                nc.tensor.matmul(out, w_lo[:, sl, :], m_hi[:, sl, :],
                                 start=False, stop=False, perf_mode=DR)
                nc.tensor.matmul(out, w_hi[:, sl, :], m_lo[:, sl, :],
                                 start=False, stop=stop, perf_mode=DR)

            def rope(dst, src_ps, tb):
                """dst[:] = rope(src_ps) for one head's [H, TB] block (fp16).
                Half-swap via SBUF->SBUF DMA partition reorder; the sign of
                the swapped half is folded into the sin table ([-sin; +sin])."""
                cs = cos_sb[:, tb * TB:(tb + 1) * TB]
                sn = sin_sb[:, tb * TB:(tb + 1) * TB]
                raw = rp.tile([P, TB], fp16, tag="raw", name="raw")
                nc.scalar.copy(raw[:], src_ps[:])
                shuf = rp.tile([P, TB], fp16, tag="shuf", name="shuf")
                nc.sync.dma_start(shuf[0:P // 2, :], raw[P // 2:P, :])
                nc.sync.dma_start(shuf[P // 2:P, :], raw[0:P // 2, :])
                tmp = rp.tile([P, TB], fp16, tag="tmp", name="tmp")
                nc.vector.tensor_tensor(dst, raw[:], cs, Alu.mult)
                nc.vector.tensor_tensor(tmp[:], shuf[:], sn, Alu.mult)
                nc.vector.tensor_tensor(dst, dst, tmp[:], Alu.add)

            def kv_k_chunks(tbx, split=False):
                """k-projection chunk closures (+ psk tiles) for tbx. With
                split=True, returns hi-closures (hi@hi terms only) followed
                by lo-closures (compensation terms), so the prologue can
                start on the hi DMAs alone."""
                x_hi, x_lo = x_tiles[tbx]
                psk = [ps.tile([P, TB], f32, tag="big", bufs=6,
                               name=f"psk{_k}") for _k in range(NKV)]

                def mk(p0, p1):
                    def emit():
                        for pr in range(p0, p1):
                            for kk in range(NKV):
                                mm3(psk[kk][:],
                                    wk_hi[:, :, kk, :], wk_lo[:, :, kk, :],
                                    x_hi, x_lo, pr,
                                    start=pr == 0, stop=pr == NDP - 1)
                    return emit

                def mk_hi(p0, p1):
                    def emit():
                        for pr in range(p0, p1):
                            sl = slice(2 * pr, 2 * pr + 2)
                            for kk in range(NKV):
                                nc.tensor.matmul(
                                    psk[kk][:], wk_hi[:, sl, kk, :],
                                    x_hi[:, sl, :], start=pr == 0,
                                    stop=False, perf_mode=DR)
                    return emit

                def mk_lo(p0, p1):
                    def emit():
                        for pr in range(p0, p1):
                            sl = slice(2 * pr, 2 * pr + 2)
                            for kk in range(NKV):
                                nc.tensor.matmul(
                                    psk[kk][:], wk_lo[:, sl, kk, :],
                                    x_hi[:, sl, :], start=False,
                                    stop=False, perf_mode=DR)
                                nc.tensor.matmul(
                                    psk[kk][:], wk_hi[:, sl, kk, :],
                                    x_lo[:, sl, :], start=False,
                                    stop=pr == NDP - 1 and kk == NKV - 1,
                                    perf_mode=DR)
                    return emit

                if split:
                    cls = ([mk_hi(p, p + 2) for p in range(0, NDP, 2)] +
                           [mk_lo(p, p + 2) for p in range(0, NDP, 2)])
                    return cls, psk
                return [mk(p, p + 2) for p in range(0, NDP, 2)], psk

            def kv_v_chunks(tbx):
                """v-projection closures for tbx. Each of the 4 t-chunks gets
                its OWN PSUM tile, sequenced so a chunk is copied out to
                v_all before its bank slot is reused."""
                x_hi, x_lo = x_tiles[tbx]
                state = {}

                def mk_mm(c, p0, p1):
                    def emit():
                        if p0 == 0:
                            state[c] = ps.tile([P, NKV * H], f32, tag="big",
                                               bufs=6, name=f"psv{c}")
                        csl = slice(c * P, (c + 1) * P)
                        for pr in range(p0, p1):
                            mm3(state[c][:],
                                x_hi[:, :, csl], x_lo[:, :, csl],
                                wv_hi, wv_lo, pr,
                                start=pr == 0, stop=pr == NDP - 1)
                    return emit

                def mk_copy(c):
                    def emit():
                        nc.scalar.copy(
                            v_all[:, :, tbx * TC + c, :],
                            state[c][:].rearrange("p (h e) -> p h e", h=NKV))
                    return emit

                chunks = []
                for c in (0, 2):
                    chunks += [mk_mm(c, p, p + 4) for p in range(0, NDP, 4)]
                chunks.append(mk_copy(0))
                chunks += [mk_mm(1, p, p + 4) for p in range(0, NDP, 4)]
                chunks.append(mk_copy(2))
                chunks += [mk_mm(3, p, p + 4) for p in range(0, NDP, 4)]
                chunks.append(mk_copy(1))
                chunks.append(mk_copy(3))
                return chunks, None

            def kv_finish_k(tbx, psk):
                tsl = slice(tbx * TB, (tbx + 1) * TB)
                for kk in range(NKV):
                    rope(kT_all[:, kk, tsl], psk[kk], tbx)

            wqs = {}
            psqs = {}

            def load_wq(h, tb):
                wq_hi = wqp.tile([P, DT, H], fp8, tag="wqh", name=f"wqh{h}")
                wq_lo = wqp.tile([P, DT, H], fp8, tag="wql", name=f"wql{h}")
                for c in range(2):
                    dsl = slice(c * 16 * P, (c + 1) * 16 * P)
                    csl = slice(c * 16, (c + 1) * 16)
                    nc.sync.dma_start(
                        wq_hi[:, csl, :],
                        wqh[dsl, h, :].rearrange("(dt p) e -> p dt e", p=P))
                    nc.sync.dma_start(
                        wq_lo[:, csl, :],
                        wql[dsl, h, :].rearrange("(dt p) e -> p dt e", p=P))
                wqs[h] = (wq_hi, wq_lo)

            def qchunks(h, tb):
                """Closures each emitting a few of head h's 48 accumulating
                q-projection DoubleRow matmuls (wq must already be loading)."""
                x_hi, x_lo = x_tiles[tb]
                wq_hi, wq_lo = wqs[h]
                psq = ps.tile([P, TB], f32, tag="big", bufs=6, name=f"psq{h}")
                psqs[h] = psq

                def mk(p0, p1):
                    def emit():
                        for pr in range(p0, p1):
                            mm3(psq[:], wq_hi, wq_lo, x_hi, x_lo, pr,
                                start=pr == 0, stop=pr == NDP - 1)
                    return emit

                return [mk(p, p + 2) for p in range(0, NDP, 2)]

            def attn(h, tb, qt, hp, fills, tail_fn, pe_fills=True,
                     head_fn=None, fin_prev=None):
                """Attention for head h. `fills` are closures interleaved into
                the s-block loop (a later head's q matmuls, or DMA prefetch);
                `tail_fn` (that head's rope) is emitted once fills are done."""
                nsb = TC * (tb + 1)
                ndg = nsb - TC  # non-diagonal s-block count
                kk = h // 4
                enc_ps = ps.tile([P, TB], f32, tag="acc", bufs=2,
                                 name=f"encps{h}")
                exacc = eap.tile([P, TB], fp16, tag="eacc", name=f"eacc{h}")
                look = 5
                ex = [None] * nsb
                tail_state = {"done": tail_fn is None}

                def pre(sb):
                    r = sb - ndg
                    off = P * r if r >= 0 else 0
                    csl = slice(off, TB)
                    lg = ps.tile([P, TB], f32, tag="big", bufs=6,
                                 name=f"lg{sb}")
                    nc.tensor.matmul(lg[:, csl],
                                     kT_all[:, kk, sb * P:(sb + 1) * P],
                                     qt[:, csl])
                    ex_t = ep.tile([P, TB], fp16, tag="ex", name=f"ex{sb}")
                    nc.scalar.activation(ex_t[:, csl], lg[:, csl], Act.Exp,
                                         scale=EXP_SCALE)
                    if r >= 0:
                        nc.vector.tensor_tensor(ex_t[:, off:off + P],
                                                ex_t[:, off:off + P],
                                                tri_sb[:], Alu.mult)
                    ex[sb] = (ex_t, csl)

                def post(sb):
                    ex_t, csl = ex[sb]
                    nc.tensor.matmul(enc_ps[:, csl], v_all[:, kk, sb, :],
                                     ex_t[:, csl],
                                     start=sb == 0, stop=sb == nsb - 1)
                    # exp-tile accumulation for the softmax denominator (DVE)
                    if sb == 0:
                        nc.vector.tensor_copy(exacc[:], ex_t[:])
                    else:
                        nc.vector.tensor_tensor(exacc[:, csl], exacc[:, csl],
                                                ex_t[:, csl], Alu.add)

                ci = 0
                while ci < min(2, len(fills)):
                    fills[ci]()
                    ci += 1
                if head_fn is not None:
                    head_fn()
                for sb in range(min(look, nsb)):
                    pre(sb)
                if fin_prev is not None:
                    fin_prev()
                for sb in range(nsb):
                    want = min(len(fills),
                               max((sb + 3) * len(fills) // nsb, 3))
                    while ci < want:
                        fills[ci]()
                        ci += 1
                    if ci == len(fills) and not tail_state["done"]:
                        tail_fn()
                        tail_state["done"] = True
                    post(sb)
                    if sb + look < nsb:
                        pre(sb + look)
                while ci < len(fills):
                    fills[ci]()
                    ci += 1
                if not tail_state["done"]:
                    tail_fn()

                def finalize():
                    lsum = lp.tile([P, TB], f32, tag="lsum", name="lsum")
                    nc.gpsimd.partition_all_reduce(lsum[:], exacc[:], P,
                                                   bass_isa.ReduceOp.add)
                    rinv = lp.tile([P, TB], f32, tag="rinv", name="rinv")
                    nc.vector.reciprocal(rinv[:], lsum[:])
                    tmp = lp.tile([P, TB], f32, tag="etmp", name="etmp")
                    nc.vector.tensor_tensor(tmp[:], enc_ps[:], rinv[:],
                                            Alu.mult)
                    pr, parity = h // 2, h % 2
                    hi_sl = enc_hi[pr][:, parity, :]
                    nc.scalar.copy(hi_sl, tmp[:])
                    nc.vector.tensor_tensor(enc_lo[pr][:, parity, :],
                                            tmp[:], hi_sl, Alu.subtract)
                return finalize

            wo_tiles = {}

            def load_wo(dc):
                dsl = slice(dc * TB, (dc + 1) * TB)
                wo_h = wop.tile([P, NQ, TB], fp8, tag="woh", bufs=3,
                                name=f"woh{dc}")
                nc.sync.dma_start(wo_h[:], woh[:, :, dsl])
                wo_l = wop.tile([P, NQ, TB], fp8, tag="wol", bufs=3,
                                name=f"wol{dc}")
                nc.sync.dma_start(wo_l[:], wol[:, :, dsl])
                wo_tiles[dc] = (wo_h, wo_l)

            def oproj_pair(yps, pr, dc, tci, start, stop):
                """o-projection matmuls for head pair pr into yps.
                3-term compensated, or single hi@hi for the direct pair on
                the direct d-chunks."""
                wo_h, wo_l = wo_tiles[dc]
                tsl = slice(tci * P, (tci + 1) * P)
                psl = slice(2 * pr, 2 * pr + 2)
                e_hi = enc_hi[pr][:, :, tsl]
                direct = pr == DIRECT_PAIR and dc in DIRECT_DCS
                nc.tensor.matmul(yps, e_hi, wo_h[:, psl, :], start=start,
                                 stop=stop and direct, perf_mode=DR)
                if not direct:
                    nc.tensor.matmul(yps, e_hi, wo_l[:, psl, :], start=False,
                                     stop=False, perf_mode=DR)
                    nc.tensor.matmul(yps, enc_lo[pr][:, :, tsl],
                                     wo_h[:, psl, :], start=False, stop=stop,
                                     perf_mode=DR)

            def ph3(tb, nxt, pp=None):
                for dc in range(NDC):
                    if nxt is not None and dc == 4:
                        load_wq(2, nxt)
                    if nxt is not None and dc == 6:
                        load_wq(3, nxt)
                    if 2 <= dc + 2 < NDC:
                        load_wo(dc + 2)
                    # pairs 0..2 for all 4 t-chunks first; the last pair
                    # (DIRECT_PAIR, whose enc depends on the final fin chain)
                    # is deferred so fin's latency hides behind PE work
                    ytiles = {}
                    for tci in range(TC):
                        if pp and dc == 0 and tci in pp:
                            ytiles[tci] = pp[tci]  # pairs 0..2 already done
                            continue
                        yps = ps.tile([P, TB], f32, tag="big", bufs=6,
                                      name=f"yps{dc}_{tci}")
                        ytiles[tci] = yps
                        for pr in range(3):
                            oproj_pair(yps[:], pr, dc, tci,
                                       start=pr == 0, stop=False)
                    for tci in range(TC):
                        oproj_pair(ytiles[tci][:], DIRECT_PAIR, dc, tci,
                                   start=False, stop=True)
                        ys = yp.tile([P, TB], bf16, tag="ys", bufs=3,
                                     name=f"ys{dc}_{tci}")
                        nc.scalar.copy(ys[:], ytiles[tci][:])
                        nc.sync.dma_start(
                            y[tb * TB + tci * P:tb * TB + (tci + 1) * P,
                              dc * TB:(dc + 1) * TB], ys[:])

            # ---- startup: hi-phase first (k hi@hi streams on the hi DMAs
            # alone), then lo/v phase ----
            xh0 = xp.tile([P, DT, TB], fp8, tag="xh", name="xh0")
            xl0 = xp.tile([P, DT, TB], fp8, tag="xl", name="xl0")
            x_tiles = {0: (xh0, xl0)}

            def _xdma(dst, src, g):
                nc.sync.dma_start(
                    dst[:, 4 * g:4 * (g + 1), :],
                    src[4 * g * P:4 * (g + 1) * P, 0:TB]
                    .rearrange("(g p) t -> p g t", p=P))

            kcs, psk0 = kv_k_chunks(0, split=True)  # 8 hi + 8 lo closures
            vcl, _ = kv_v_chunks(0)

            nc.sync.dma_start(wk_hi[:, 0:4], wkh_r[:, 0:4])
            _xdma(xh0, xhi, 0)
            for g in range(1, 8):
                c4 = slice(4 * g, 4 * (g + 1))
                nc.sync.dma_start(wk_hi[:, c4], wkh_r[:, c4])
                _xdma(xh0, xhi, g)
                kcs[g - 1]()
            kcs[7]()

            nc.sync.dma_start(cos_sb[:, 0:TB], cos[:, 0:TB])
            nc.sync.dma_start(sin_sb[:, 0:TB], sin[:, 0:TB])
            nc.sync.dma_start(tri_sb[:], tri[:])
            for g in range(8):
                c4 = slice(4 * g, 4 * (g + 1))
                nc.sync.dma_start(wk_lo[:, c4], wkl_r[:, c4])
                _xdma(xl0, xlo, g)
                nc.sync.dma_start(wv_hi[:, c4], wvh_r[:, c4])
                nc.sync.dma_start(wv_lo[:, c4], wvl_r[:, c4])
                if g == 2:
                    load_wq(0, 0)
                if g == 3:
                    load_wq(1, 0)
                if g >= 1:
                    kcs[8 + g - 1]()
            kcs[15]()
            vcl[0]()
            vcl[1]()
            kv_finish_k(0, psk0)
            for b in vcl[2:]:
                b()

            kv_state = {}

            def mk_rope(j, qts, tb):
                qts[j] = qtp.tile([P, TB], fp16, tag="qt", name=f"qt{j}")

                def tail():
                    rope(qts[j], psqs[j], tb)
                return tail

            for tb in range(NTB):
                enc_hi = [encp.tile([P, 2, TB], fp8, tag=f"ehi{pr}",
                                    name=f"ehi{pr}") for pr in range(4)]
                enc_lo = [encp.tile([P, 2, TB], fp8, tag=f"elo{pr}",
                                    name=f"elo{pr}") for pr in range(4)]
                qts = {}

                # heads 0 and 1: dense q-passes up front (depth-2 priming);
                # rope(0) after both passes, rope(1) deferred into attn(0)
                if tb == 0:
                    load_wq(2, tb)
                for ch in qchunks(0, tb):
                    ch()
                for ch in qchunks(1, tb):
                    ch()
                mk_rope(0, qts, tb)()
                rope1_fn = mk_rope(1, qts, tb)

                nxt = tb + 1 if tb + 1 < NTB else None
                fin = None
                pp = {}

                def pp_mm(tci, pa, pb, pp=pp):
                    def emit():
                        if tci not in pp:
                            pp[tci] = ps.tile([P, TB], f32, tag="big",
                                              bufs=6, name=f"ypsP{tci}")
                        for pr in range(pa, pb):
                            oproj_pair(pp[tci][:], pr, 0, tci,
                                       start=pr == 0, stop=False)
                    return emit

                for h in range(NQ):
                    if h + 3 < NQ and (tb == 0 or h >= 1):
                        load_wq(h + 3, tb)
                    tail_fn = None
                    pe_fills = True
                    if h + 2 < NQ:
                        fills = qchunks(h + 2, tb)
                        tail_fn = mk_rope(h + 2, qts, tb)
                        if nxt is not None and h == 4:
                            # interleave next t-block's x prefetch
                            x_tiles[nxt] = (
                                xp.tile([P, DT, TB], fp8, tag="xh",
                                        name=f"xh{nxt}"),
                                xp.tile([P, DT, TB], fp8, tag="xl",
                                        name=f"xl{nxt}"))
                            xcl = x_dma_closures(nxt, *x_tiles[nxt], nch=4)
                            merged = []
                            for i in range(max(len(fills), len(xcl))):
                                if i < len(fills):
                                    merged.append(fills[i])
                                if i < len(xcl):
                                    merged.append(xcl[i])
                            fills = merged
                        elif nxt is not None and h == 5:
                            fills = fills + [
                                lambda tb=tb: load_wq(0, tb + 1),
                                lambda tb=tb: load_wq(1, tb + 1)]
                    elif h == NQ - 2:
                        # next t-block's k projections + first wo loads
                        fills = [lambda dc=dc: load_wo(dc) for dc in range(2)]
                        if nxt is not None:
                            kc, psk_n = kv_k_chunks(nxt)
                            kv_state["psk"] = psk_n
                            fills = kc + fills
                        else:
                            # last t-block: pre-accumulate pairs 0..2 of
                            # ph3's first d-chunk to shorten the tail
                            fills += [pp_mm(0, 0, 2), pp_mm(1, 0, 2),
                                      pp_mm(0, 2, 3), pp_mm(1, 2, 3)]
                    else:
                        # last head: next t-block's v projections, k rope,
                        # v copies
                        fills = []
                        if nxt is not None:
                            vc, _ = kv_v_chunks(nxt)
                            fills = ([vc[0],
                                      lambda: kv_finish_k(nxt,
                                                          kv_state["psk"])] +
                                     vc[1:])
                        else:
                            pe_fills = False
                    fin = attn(h, tb, qts[h], h // 2, fills, tail_fn,
                               pe_fills, head_fn=rope1_fn if h == 0 else None,
                               fin_prev=fin)
                fin()
                ph3(tb, nxt, pp)

    nc.compile()
    return nc


def _get_nc():
    if "nc" not in _STATE:
        _STATE["nc"] = _build_nc()
    return _STATE["nc"]


def _q8(a):
    return np.ascontiguousarray(a, dtype=np.float32).astype(F8)


def _hilo(a):
    hi = _q8(a)
    lo = _q8(np.asarray(a, np.float32) - hi.astype(np.float32))
    return hi, lo


def _make_in_maps(x, positions, wq, wkv, wo):
    """Build the 8 per-core input dicts (host-side quantization + tables)."""
    B = x.shape[0]
    in_maps = []

    tables = []
    for b in range(B):
        pos = np.asarray(positions[b], np.float64)
        timescale = 10000.0 ** ((2.0 / H) * np.arange(H // 2))
        rad = pos[:, None] / timescale[None, :]          # [T, H/2]
        c64 = np.cos(rad).T                              # [H/2, T]
        s64 = np.sin(rad).T
        tables.append((
            np.ascontiguousarray(np.concatenate([c64, c64], 0)).astype(F16),
            np.ascontiguousarray(np.concatenate([-s64, s64], 0)).astype(F16),
        ))

    xThilo = [_hilo(np.ascontiguousarray(x[b].T)) for b in range(B)]

    i = np.arange(P)[:, None]
    j = np.arange(P)[None, :]
    tri = np.ascontiguousarray((j >= i).astype(F16))

    for c in range(8):
        b, hg = c // 4, c % 4
        qs = slice(NQ * hg, NQ * (hg + 1))
        ks = slice(NKV * hg, NKV * (hg + 1))
        cos_t, sin_t = tables[b]
        xh, xl = xThilo[b]
        wq_h, wq_l = _hilo(SW_QK * wq[qs].transpose(1, 0, 2))
        wk_h, wk_l = _hilo(SW_QK * wkv[0, ks].transpose(1, 0, 2))
        wv_h, wv_l = _hilo(SW_V * wkv[1, ks].transpose(1, 0, 2))
        # wo: [8, H, D] -> [H, 8, D]; adjacent heads form DoubleRow pairs
        wo_h, wo_l = _hilo((SW_O * wo[qs]).transpose(1, 0, 2))
        in_maps.append({
            "xhi": xh,
            "xlo": xl,
            "wqh": wq_h,
            "wql": wq_l,
            "wkh": wk_h,
            "wkl": wk_l,
            "wvh": wv_h,
            "wvl": wv_l,
            "woh": wo_h,
            "wol": wo_l,
            "cos": cos_t,
            "sin": sin_t,
            "tri": tri,
        })
    return in_maps


def run_cores(in_maps, trace=False, trace_cores=None):
    from concourse.bass_utils import run_bass_kernel_spmd
    nc = _get_nc()
    kw = {}
    if trace:
        kw = dict(trace=True,
                  trace_cores=trace_cores or list(range(8)))
    return run_bass_kernel_spmd(nc, in_maps, core_ids=list(range(8)), **kw)


def kernel(**inputs):
    x = np.asarray(inputs["x"], np.float32)
    positions = np.asarray(inputs["positions"])
    wq = np.asarray(inputs["wq"], np.float32)
    wkv = np.asarray(inputs["wkv"], np.float32)
    wo = np.asarray(inputs["wo"], np.float32)
    B = x.shape[0]
    assert x.shape == (2, T, D) and wq.shape == (32, D, H)

    in_maps = _make_in_maps(x, positions, wq, wkv, wo)
    res = run_cores(in_maps)
    y = np.zeros((B, T, D), np.float32)
    inv = 1.0 / (SW_V * SW_O)
    for c, r in enumerate(res.results):
        y[c // 4] += np.asarray(r["y"], np.float32) * inv
    return y


if __name__ == "__main__":
    _build_nc()
    print("build OK")


# revision 31
# speedup vs baseline: 1.1887x; 1.0821x over previous
"""Trainium2 Bass kernel for GQA attention prefill (B=2, T=2048, D=4096, N=32, K=8, H=128).

Sharding: 8 cores = 2 (batch) x 4 (head-groups). Each core handles one batch
element, 8 q-heads and its 2 kv-heads, producing a partial output projection
(summed over its heads). Host sums the 4 partials per batch element (and
undoes the x512 weight scaling).

Precision scheme (PE cost model: bf16/fp16 1.0 cycles/row, fp8+DoubleRow 0.5
cycles/row with a 256-deep contraction -> 4x effective throughput):
  - q/k/v/o projections run as fp8 DoubleRow with hi+lo error compensation:
    w ~ whi + wlo, x ~ xhi + xlo (each e4m3), y = whi@xhi + wlo@xhi + whi@xlo.
    3 quarter-cost matmuls = 0.75x the bf16 cost at ~0.1% error. Weights are
    pre-scaled into e4m3's normal range (wq,wk x64 folded into the exp scale;
    wv x16 cancels against the softmax 1/l fold; wo x32 undone on host).
  - one o-proj head-pair runs direct fp8 (1 matmul, 0.25x cost), spending the
    correctness headroom (~1.7% of final norm).
  - attention (rope, logits, exp, AV) runs in fp16: same PE cost as bf16,
    ~8x lower noise.

Per-core pipeline, software-pipelined per head so PE never idles:
  passA(tb):  k,v projections from xhi/xlo (DMA'd once per t-block, resident
              in SBUF); rope(k) via SBUF->SBUF DMA half-swap plus DVE
              elementwise with fp16 cos/sin tables ([-sin; sin] fold).
  per head h: q-projection matmuls for head h+2 are emitted interleaved with
              head h's attention s-block loop. Attention: logitsT [s128,t<=512]
              = kT-block @ qt (fp16), exp on ACT (scale absorbs the x64 weight
              scales), 0/1 triangle mask multiply on DVE for diagonal tiles,
              AV accumulates in PSUM; denominators accumulate on DVE in f32.
  fin(h):     gpsimd partition reduce -> reciprocal -> DVE psum*rinv -> f32
              tmp, then ACT copy -> enc_hi (fp8) and DVE sub -> enc_lo (fp8),
              pair-interleaved for the o-proj stationary operand.
  ph3(tb):    output projection from enc pair tiles: 3 pairs x 3-term + 1
              direct pair = 10 DoubleRow matmuls per (dchunk, tchunk); PSUM ->
              bf16 SBUF copies on ACT, DMA out per 512-wide d-chunk.
"""

import os
import sys

import numpy as np

for _p in ("/opt/trn_rl_repo", "/root/.axon_site/_ro/trn_rl_repo"):
    if _p not in sys.path and os.path.isdir(_p):
        sys.path.append(_p)

import ml_dtypes

BF16 = ml_dtypes.bfloat16
F16 = np.float16
F8 = ml_dtypes.float8_e4m3fn

P = 128
T = 2048
D = 4096
H = 128
NQ = 8   # q heads per core
NKV = 2  # kv heads per core
TB = 512
NTB = T // TB        # 4
DT = D // P          # 32 d-tiles
NDP = DT // 2        # 16 d-tile pairs
NSB = T // P         # 16 s-blocks
TC = TB // P         # 4 t-chunks per t-block
NDC = D // TB        # 8 d-chunks for the output projection
SCALE = float(H) ** -0.5
SW_QK = 64.0         # wq/wk host scale (folded into exp scale)
SW_V = 16.0          # wv host scale (cancels vs softmax 1/l fold)
SW_O = 32.0          # wo host scale (undone on host with 1/(SW_V*SW_O))
EXP_SCALE = SCALE / (SW_QK * SW_QK)
DIRECT_PAIR = 3      # o-proj head pair computed hi@hi only ...
DIRECT_DCS = frozenset({0, 1, 2, 3})  # ... on these 512-wide d-chunks

_STATE = {}


def _build_nc():
    import concourse.mybir as mybir
    import concourse.tile as tile
    from concourse import bacc
    from concourse import bass_isa

    f32 = mybir.dt.float32
    fp16 = mybir.dt.float16
    fp8 = mybir.dt.float8e4
    bf16 = mybir.dt.bfloat16
    Alu = mybir.AluOpType
    Act = mybir.ActivationFunctionType
    DR = mybir.MatmulPerfMode.DoubleRow

    nc = bacc.Bacc(None, target_bir_lowering=False, debug=False)

    xhi = nc.dram_tensor("xhi", [D, T], fp8, kind="ExternalInput")
    xlo = nc.dram_tensor("xlo", [D, T], fp8, kind="ExternalInput")
    # weights are partition-major and hi/lo-packed so each load is one
    # fully-contiguous DMA (>=512B runs avoid the half-bandwidth penalty):
    # wq8[h, p] = [NDP, 2(hi/lo), 2(pair), H], wk8/wv8[p] likewise
    wq8 = nc.dram_tensor("wq8", [NQ, P, 2, NDP, 2, H], fp8,
                         kind="ExternalInput")
    wk8 = nc.dram_tensor("wk8", [P, 2, NDP, 2, NKV, H], fp8,
                         kind="ExternalInput")
    wv8 = nc.dram_tensor("wv8", [P, 2, NDP, 2, NKV * H], fp8,
                         kind="ExternalInput")
    # o-projection weights, head-major within rows: [H, NQ, D] (adjacent
    # heads form the DoubleRow pairs)
    woh = nc.dram_tensor("woh", [H, NQ, D], fp8, kind="ExternalInput")
    wol = nc.dram_tensor("wol", [H, NQ, D], fp8, kind="ExternalInput")
    cos = nc.dram_tensor("cos", [P, T], fp16, kind="ExternalInput")
    sin = nc.dram_tensor("sin", [P, T], fp16, kind="ExternalInput")
    tri = nc.dram_tensor("tri", [P, P], fp16, kind="ExternalInput")
    y = nc.dram_tensor("y", [T, D], bf16, kind="ExternalOutput")

    with tile.TileContext(nc) as tc:
        with (
            tc.tile_pool(name="const", bufs=1) as const,
            tc.tile_pool(name="xp", bufs=2) as xp,
            tc.tile_pool(name="wqp", bufs=2) as wqp,
            tc.tile_pool(name="qtp", bufs=3) as qtp,
            tc.tile_pool(name="rp", bufs=3) as rp,
            tc.tile_pool(name="ep", bufs=6) as ep,
            tc.tile_pool(name="eap", bufs=2) as eap,
            tc.tile_pool(name="encp", bufs=1) as encp,
            tc.tile_pool(name="lp", bufs=1) as lp,
            tc.tile_pool(name="wop", bufs=2) as wop,
            tc.tile_pool(name="yp", bufs=2) as yp,
            tc.tile_pool(name="ps", bufs=1, space="PSUM") as ps,
        ):
            wk_t = const.tile([P, 2, NDP, 2, NKV, H], fp8, tag="wk")
            wv_t = const.tile([P, 2, NDP, 2, NKV * H], fp8, tag="wv")
            tri_sb = const.tile([P, P], fp16, tag="tri")
            cos_sb = const.tile([P, T], fp16, tag="cos")
            sin_sb = const.tile([P, T], fp16, tag="sin")
            kT_all = const.tile([P, NKV, T], fp16, tag="kT")
            v_all = const.tile([P, NKV, NSB, H], fp16, tag="v")

            def x_dma_closures(tb, x_hi, x_lo, tables=True, nch=8):
                """nch closures, each DMA-ing a DT/nch-d-tile chunk of
                xhi+xlo for tb (plus this t-block's cos/sin on chunk 0)."""
                tsl = slice(tb * TB, (tb + 1) * TB)
                step = DT // nch

                def mk(c8):
                    def emit():
                        dsl = slice(c8 * step * P, (c8 + 1) * step * P)
                        csl = slice(c8 * step, (c8 + 1) * step)
                        nc.sync.dma_start(
                            x_hi[:, csl, :],
                            xhi[dsl, tsl].rearrange("(g p) t -> p g t", p=P))
                        nc.sync.dma_start(
                            x_lo[:, csl, :],
                            xlo[dsl, tsl].rearrange("(g p) t -> p g t", p=P))
                        if c8 == 0 and tables:
                            nc.sync.dma_start(cos_sb[:, tsl], cos[:, tsl])
                            nc.sync.dma_start(sin_sb[:, tsl], sin[:, tsl])
                    return emit

                return [mk(c8) for c8 in range(nch)]

            def mm3(out, wf, mf, pr, start, stop):
                """3-term compensated DoubleRow accumulation for d-pair pr.
                wf(pr, s) -> [P, 2, M] stationary slice, mf(pr, s) ->
                [P, 2, N] moving slice (s: 0=hi, 1=lo)."""
                nc.tensor.matmul(out, wf(pr, 0), mf(pr, 0),
                                 start=start, stop=False, perf_mode=DR)
                nc.tensor.matmul(out, wf(pr, 1), mf(pr, 0),
                                 start=False, stop=False, perf_mode=DR)
                nc.tensor.matmul(out, wf(pr, 0), mf(pr, 1),
                                 start=False, stop=stop, perf_mode=DR)

            def rope(dst, src_ps, tb):
                """dst[:] = rope(src_ps) for one head's [H, TB] block (fp16).
                Half-swap via SBUF->SBUF DMA partition reorder; the sign of
                the swapped half is folded into the sin table ([-sin; +sin])."""
                cs = cos_sb[:, tb * TB:(tb + 1) * TB]
                sn = sin_sb[:, tb * TB:(tb + 1) * TB]
                raw = rp.tile([P, TB], fp16, tag="raw", name="raw")
                nc.scalar.copy(raw[:], src_ps[:])
                shuf = rp.tile([P, TB], fp16, tag="shuf", name="shuf")
                nc.sync.dma_start(shuf[0:P // 2, :], raw[P // 2:P, :])
                nc.sync.dma_start(shuf[P // 2:P, :], raw[0:P // 2, :])
                tmp = rp.tile([P, TB], fp16, tag="tmp", name="tmp")
                nc.vector.tensor_tensor(dst, raw[:], cs, Alu.mult)
                nc.vector.tensor_tensor(tmp[:], shuf[:], sn, Alu.mult)
                nc.vector.tensor_tensor(dst, dst, tmp[:], Alu.add)

            def kv_k_chunks(tbx, split=False):
                """k-projection chunk closures (+ psk tiles) for tbx. With
                split=True, returns hi-closures (hi@hi terms only) followed
                by lo-closures (compensation terms), so the prologue can
                start on the hi DMAs alone."""
                x_hi, x_lo = x_tiles[tbx]
                psk = [ps.tile([P, TB], f32, tag="big", bufs=6,
                               name=f"psk{_k}") for _k in range(NKV)]

                def xf(pr, s):
                    return (x_hi if s == 0 else x_lo)[:, 2 * pr:2 * pr + 2, :]

                def wf(kk):
                    return lambda pr, s: wk_t[:, s, pr, :, kk, :]

                def mk(p0, p1):
                    def emit():
                        for pr in range(p0, p1):
                            for kk in range(NKV):
                                mm3(psk[kk][:], wf(kk), xf, pr,
                                    start=pr == 0, stop=pr == NDP - 1)
                    return emit

                def mk_hi(p0, p1):
                    def emit():
                        for pr in range(p0, p1):
                            for kk in range(NKV):
                                nc.tensor.matmul(
                                    psk[kk][:], wk_t[:, 0, pr, :, kk, :],
                                    xf(pr, 0), start=pr == 0,
                                    stop=False, perf_mode=DR)
                    return emit

                def mk_lo(p0, p1):
                    def emit():
                        for pr in range(p0, p1):
                            for kk in range(NKV):
                                nc.tensor.matmul(
                                    psk[kk][:], wk_t[:, 1, pr, :, kk, :],
                                    xf(pr, 0), start=False,
                                    stop=False, perf_mode=DR)
                                nc.tensor.matmul(
                                    psk[kk][:], wk_t[:, 0, pr, :, kk, :],
                                    xf(pr, 1), start=False,
                                    stop=pr == NDP - 1 and kk == NKV - 1,
                                    perf_mode=DR)
                    return emit

                if split:
                    cls = ([mk_hi(p, p + 2) for p in range(0, NDP, 2)] +
                           [mk_lo(p, p + 2) for p in range(0, NDP, 2)])
                    return cls, psk
                return [mk(p, p + 2) for p in range(0, NDP, 2)], psk

            def kv_v_chunks(tbx):
                """v-projection closures for tbx. Each of the 4 t-chunks gets
                its OWN PSUM tile, sequenced so a chunk is copied out to
                v_all before its bank slot is reused."""
                x_hi, x_lo = x_tiles[tbx]
                state = {}

                def mk_mm(c, p0, p1):
                    def emit():
                        if p0 == 0:
                            state[c] = ps.tile([P, NKV * H], f32, tag="big",
                                               bufs=6, name=f"psv{c}")
                        csl = slice(c * P, (c + 1) * P)

                        def xf(pr, s):
                            return (x_hi if s == 0
                                    else x_lo)[:, 2 * pr:2 * pr + 2, csl]

                        for pr in range(p0, p1):
                            mm3(state[c][:], xf,
                                lambda pr, s: wv_t[:, s, pr, :, :], pr,
                                start=pr == 0, stop=pr == NDP - 1)
                    return emit

                def mk_copy(c):
                    def emit():
                        nc.scalar.copy(
                            v_all[:, :, tbx * TC + c, :],
                            state[c][:].rearrange("p (h e) -> p h e", h=NKV))
                    return emit

                chunks = []
                for c in (0, 2):
                    chunks += [mk_mm(c, p, p + 4) for p in range(0, NDP, 4)]
                chunks.append(mk_copy(0))
                chunks += [mk_mm(1, p, p + 4) for p in range(0, NDP, 4)]
                chunks.append(mk_copy(2))
                chunks += [mk_mm(3, p, p + 4) for p in range(0, NDP, 4)]
                chunks.append(mk_copy(1))
                chunks.append(mk_copy(3))
                return chunks, None

            def kv_finish_k(tbx, psk):
                tsl = slice(tbx * TB, (tbx + 1) * TB)
                for kk in range(NKV):
                    rope(kT_all[:, kk, tsl], psk[kk], tbx)

            wqs = {}
            psqs = {}

            def load_wq(h, tb):
                wq_t = wqp.tile([P, 2, NDP, 2, H], fp8, tag="wq",
                                name=f"wq{h}")
                nc.sync.dma_start(wq_t[:], wq8[h])
                wqs[h] = wq_t

            def qchunks(h, tb):
                """Closures each emitting a few of head h's 48 accumulating
                q-projection DoubleRow matmuls (wq must already be loading)."""
                x_hi, x_lo = x_tiles[tb]
                wq_t = wqs[h]
                psq = ps.tile([P, TB], f32, tag="big", bufs=6, name=f"psq{h}")
                psqs[h] = psq

                def xf(pr, s):
                    return (x_hi if s == 0 else x_lo)[:, 2 * pr:2 * pr + 2, :]

                def mk(p0, p1):
                    def emit():
                        for pr in range(p0, p1):
                            mm3(psq[:], lambda pr, s: wq_t[:, s, pr, :, :],
                                xf, pr, start=pr == 0, stop=pr == NDP - 1)
                    return emit

                return [mk(p, p + 2) for p in range(0, NDP, 2)]

            def attn(h, tb, qt, hp, fills, tail_fn, pe_fills=True,
                     head_fn=None, fin_prev=None):
                """Attention for head h. `fills` are closures interleaved into
                the s-block loop (a later head's q matmuls, or DMA prefetch);
                `tail_fn` (that head's rope) is emitted once fills are done."""
                nsb = TC * (tb + 1)
                ndg = nsb - TC  # non-diagonal s-block count
                kk = h // 4
                enc_ps = ps.tile([P, TB], f32, tag="acc", bufs=2,
                                 name=f"encps{h}")
                exacc = eap.tile([P, TB], fp16, tag="eacc", name=f"eacc{h}")
                look = 5
                ex = [None] * nsb
                tail_state = {"done": tail_fn is None}

                def pre(sb):
                    r = sb - ndg
                    off = P * r if r >= 0 else 0
                    csl = slice(off, TB)
                    lg = ps.tile([P, TB], f32, tag="big", bufs=6,
                                 name=f"lg{sb}")
                    nc.tensor.matmul(lg[:, csl],
                                     kT_all[:, kk, sb * P:(sb + 1) * P],
                                     qt[:, csl])
                    ex_t = ep.tile([P, TB], fp16, tag="ex", name=f"ex{sb}")
                    nc.scalar.activation(ex_t[:, csl], lg[:, csl], Act.Exp,
                                         scale=EXP_SCALE)
                    if r >= 0:
                        nc.vector.tensor_tensor(ex_t[:, off:off + P],
                                                ex_t[:, off:off + P],
                                                tri_sb[:], Alu.mult)
                    ex[sb] = (ex_t, csl)

                def post(sb):
                    ex_t, csl = ex[sb]
                    nc.tensor.matmul(enc_ps[:, csl], v_all[:, kk, sb, :],
                                     ex_t[:, csl],
                                     start=sb == 0, stop=sb == nsb - 1)
                    # exp-tile accumulation for the softmax denominator (DVE)
                    if sb == 0:
                        nc.vector.tensor_copy(exacc[:], ex_t[:])
                    else:
                        nc.vector.tensor_tensor(exacc[:, csl], exacc[:, csl],
                                                ex_t[:, csl], Alu.add)

                ci = 0
                while ci < min(2, len(fills)):
                    fills[ci]()
                    ci += 1
                if head_fn is not None:
                    head_fn()
                for sb in range(min(look, nsb)):
                    pre(sb)
                if fin_prev is not None:
                    fin_prev()
                for sb in range(nsb):
                    want = min(len(fills),
                               max((sb + 3) * len(fills) // nsb, 3))
                    while ci < want:
                        fills[ci]()
                        ci += 1
                    if ci == len(fills) and not tail_state["done"]:
                        tail_fn()
                        tail_state["done"] = True
                    post(sb)
                    if sb + look < nsb:
                        pre(sb + look)
                while ci < len(fills):
                    fills[ci]()
                    ci += 1
                if not tail_state["done"]:
                    tail_fn()

                def finalize():
                    lsum = lp.tile([P, TB], f32, tag="lsum", name="lsum")
                    nc.gpsimd.partition_all_reduce(lsum[:], exacc[:], P,
                                                   bass_isa.ReduceOp.add)
                    rinv = lp.tile([P, TB], f32, tag="rinv", name="rinv")
                    nc.vector.reciprocal(rinv[:], lsum[:])
                    tmp = lp.tile([P, TB], f32, tag="etmp", name="etmp")
                    nc.vector.tensor_tensor(tmp[:], enc_ps[:], rinv[:],
                                            Alu.mult)
                    pr, parity = h // 2, h % 2
                    hi_sl = enc_hi[pr][:, parity, :]
                    nc.vector.tensor_copy(hi_sl, tmp[:])
                    nc.vector.tensor_tensor(enc_lo[pr][:, parity, :],
                                            tmp[:], hi_sl, Alu.subtract)
                return finalize

            wo_tiles = {}

            def load_wo(dc):
                dsl = slice(dc * TB, (dc + 1) * TB)
                wo_h = wop.tile([P, NQ, TB], fp8, tag="woh", bufs=4,
                                name=f"woh{dc}")
                nc.sync.dma_start(wo_h[:], woh[:, :, dsl])
                wo_l = wop.tile([P, NQ, TB], fp8, tag="wol", bufs=4,
                                name=f"wol{dc}")
                nc.sync.dma_start(wo_l[:], wol[:, :, dsl])
                wo_tiles[dc] = (wo_h, wo_l)

            def oproj_pair(yps, pr, dc, tci, start, stop):
                """o-projection matmuls for head pair pr into yps.
                3-term compensated, or single hi@hi for the direct pair on
                the direct d-chunks."""
                wo_h, wo_l = wo_tiles[dc]
                tsl = slice(tci * P, (tci + 1) * P)
                psl = slice(2 * pr, 2 * pr + 2)
                e_hi = enc_hi[pr][:, :, tsl]
                direct = pr == DIRECT_PAIR and dc in DIRECT_DCS
                nc.tensor.matmul(yps, e_hi, wo_h[:, psl, :], start=start,
                                 stop=stop and direct, perf_mode=DR)
                if not direct:
                    nc.tensor.matmul(yps, e_hi, wo_l[:, psl, :], start=False,
                                     stop=False, perf_mode=DR)
                    nc.tensor.matmul(yps, enc_lo[pr][:, :, tsl],
                                     wo_h[:, psl, :], start=False, stop=stop,
                                     perf_mode=DR)

            def ph3(tb, nxt, pp=None):
                for dc in range(NDC):
                    if nxt is not None and dc == 4:
                        load_wq(2, nxt)
                    if nxt is not None and dc == 6:
                        load_wq(3, nxt)
                    if dc + 4 < NDC:
                        load_wo(dc + 4)
                    # pairs 0..2 for all 4 t-chunks first; the last pair
                    # (DIRECT_PAIR, whose enc depends on the final fin chain)
                    # is deferred so fin's latency hides behind PE work
                    ytiles = {}
                    for tci in range(TC):
                        if pp and dc == 0 and tci in pp:
                            ytiles[tci] = pp[tci]  # pairs 0..2 already done
                            continue
                        yps = ps.tile([P, TB], f32, tag="big", bufs=6,
                                      name=f"yps{dc}_{tci}")
                        ytiles[tci] = yps
                        for pr in range(3):
                            oproj_pair(yps[:], pr, dc, tci,
                                       start=pr == 0, stop=False)
                    for tci in range(TC):
                        oproj_pair(ytiles[tci][:], DIRECT_PAIR, dc, tci,
                                   start=False, stop=True)
                        ys = yp.tile([P, TB], bf16, tag="ys", bufs=3,
                                     name=f"ys{dc}_{tci}")
                        nc.scalar.copy(ys[:], ytiles[tci][:])
                        nc.sync.dma_start(
                            y[tb * TB + tci * P:tb * TB + (tci + 1) * P,
                              dc * TB:(dc + 1) * TB], ys[:])

            # ---- startup: hi-phase first (k hi@hi streams on the hi DMAs
            # alone), then lo/v phase ----
            xh0 = xp.tile([P, DT, TB], fp8, tag="xh", name="xh0")
            xl0 = xp.tile([P, DT, TB], fp8, tag="xl", name="xl0")
            x_tiles = {0: (xh0, xl0)}

            def _xdma(dst, src, g):
                nc.sync.dma_start(
                    dst[:, 4 * g:4 * (g + 1), :],
                    src[4 * g * P:4 * (g + 1) * P, 0:TB]
                    .rearrange("(g p) t -> p g t", p=P))

            kcs, psk0 = kv_k_chunks(0, split=True)  # 8 hi + 8 lo closures
            vcl, _ = kv_v_chunks(0)

            nc.sync.dma_start(wk_t[:, 0, 0:2], wk8[:, 0, 0:2])
            _xdma(xh0, xhi, 0)
            for g in range(1, 8):
                p2 = slice(2 * g, 2 * (g + 1))
                nc.sync.dma_start(wk_t[:, 0, p2], wk8[:, 0, p2])
                _xdma(xh0, xhi, g)
                kcs[g - 1]()
            kcs[7]()

            nc.sync.dma_start(cos_sb[:, 0:TB], cos[:, 0:TB])
            nc.sync.dma_start(sin_sb[:, 0:TB], sin[:, 0:TB])
            nc.sync.dma_start(tri_sb[:], tri[:])
            for g in range(8):
                p2 = slice(2 * g, 2 * (g + 1))
                nc.sync.dma_start(wk_t[:, 1, p2], wk8[:, 1, p2])
                _xdma(xl0, xlo, g)
                nc.sync.dma_start(wv_t[:, :, p2], wv8[:, :, p2])
                if g == 2:
                    load_wq(0, 0)
                if g == 3:
                    load_wq(1, 0)
                if g >= 1:
                    kcs[8 + g - 1]()
            kcs[15]()
            vcl[0]()
            vcl[1]()
            kv_finish_k(0, psk0)
            for b in vcl[2:]:
                b()

            kv_state = {}

            def mk_rope(j, qts, tb):
                qts[j] = qtp.tile([P, TB], fp16, tag="qt", name=f"qt{j}")

                def tail():
                    rope(qts[j], psqs[j], tb)
                return tail

            for tb in range(NTB):
                enc_hi = [encp.tile([P, 2, TB], fp8, tag=f"ehi{pr}",
                                    name=f"ehi{pr}") for pr in range(4)]
                enc_lo = [encp.tile([P, 2, TB], fp8, tag=f"elo{pr}",
                                    name=f"elo{pr}") for pr in range(4)]
                qts = {}

                # heads 0 and 1: dense q-passes up front (depth-2 priming);
                # rope(0) after both passes, rope(1) deferred into attn(0)
                if tb == 0:
                    load_wq(2, tb)
                for ch in qchunks(0, tb):
                    ch()
                for ch in qchunks(1, tb):
                    ch()
                mk_rope(0, qts, tb)()
                rope1_fn = mk_rope(1, qts, tb)

                nxt = tb + 1 if tb + 1 < NTB else None
                fin = None
                pp = {}

                def pp_mm(tci, pa, pb, pp=pp):
                    def emit():
                        if tci not in pp:
                            pp[tci] = ps.tile([P, TB], f32, tag="big",
                                              bufs=6, name=f"ypsP{tci}")
                        for pr in range(pa, pb):
                            oproj_pair(pp[tci][:], pr, 0, tci,
                                       start=pr == 0, stop=False)
                    return emit

                for h in range(NQ):
                    if h + 3 < NQ and (tb == 0 or h >= 1):
                        load_wq(h + 3, tb)
                    tail_fn = None
                    pe_fills = True
                    if h + 2 < NQ:
                        fills = qchunks(h + 2, tb)
                        tail_fn = mk_rope(h + 2, qts, tb)
                        if nxt is not None and h == 4:
                            # interleave next t-block's x prefetch
                            x_tiles[nxt] = (
                                xp.tile([P, DT, TB], fp8, tag="xh",
                                        name=f"xh{nxt}"),
                                xp.tile([P, DT, TB], fp8, tag="xl",
                                        name=f"xl{nxt}"))
                            xcl = x_dma_closures(nxt, *x_tiles[nxt], nch=4)
                            merged = []
                            for i in range(max(len(fills), len(xcl))):
                                if i < len(fills):
                                    merged.append(fills[i])
                                if i < len(xcl):
                                    merged.append(xcl[i])
                            fills = merged
                        elif nxt is not None and h == 5:
                            fills = fills + [
                                lambda tb=tb: load_wq(0, tb + 1),
                                lambda tb=tb: load_wq(1, tb + 1)]
                    elif h == NQ - 2:
                        # next t-block's k projections + first wo loads
                        fills = [lambda dc=dc: load_wo(dc) for dc in range(2)]
                        if nxt is not None:
                            kc, psk_n = kv_k_chunks(nxt)
                            kv_state["psk"] = psk_n
                            fills = kc + fills
                        else:
                            # last t-block: pre-accumulate pairs 0..2 of
                            # ph3's first d-chunk to shorten the tail
                            fills += [pp_mm(0, 0, 2), pp_mm(1, 0, 2),
                                      pp_mm(0, 2, 3), pp_mm(1, 2, 3)]
                    else:
                        # last head: wo prefetch, then next t-block's v
                        # projections, k rope, v copies
                        fills = [lambda: load_wo(2), lambda: load_wo(3)]
                        if nxt is not None:
                            vc, _ = kv_v_chunks(nxt)
                            fills += ([vc[0],
                                       lambda: kv_finish_k(nxt,
                                                           kv_state["psk"])] +
                                      vc[1:])
                        else:
                            pe_fills = False
                    fin = attn(h, tb, qts[h], h // 2, fills, tail_fn,
                               pe_fills, head_fn=rope1_fn if h == 0 else None,
                               fin_prev=fin)
                fin()
                ph3(tb, nxt, pp)

    nc.compile()
    return nc


def _get_nc():
    if "nc" not in _STATE:
        _STATE["nc"] = _build_nc()
    return _STATE["nc"]


def _q8(a):
    return np.ascontiguousarray(a, dtype=np.float32).astype(F8)


def _hilo(a):
    hi = _q8(a)
    lo = _q8(np.asarray(a, np.float32) - hi.astype(np.float32))
    return hi, lo


def _make_in_maps(x, positions, wq, wkv, wo):
    """Build the 8 per-core input dicts (host-side quantization + tables)."""
    B = x.shape[0]
    in_maps = []

    tables = []
    for b in range(B):
        pos = np.asarray(positions[b], np.float64)
        timescale = 10000.0 ** ((2.0 / H) * np.arange(H // 2))
        rad = pos[:, None] / timescale[None, :]          # [T, H/2]
        c64 = np.cos(rad).T                              # [H/2, T]
        s64 = np.sin(rad).T
        tables.append((
            np.ascontiguousarray(np.concatenate([c64, c64], 0)).astype(F16),
            np.ascontiguousarray(np.concatenate([-s64, s64], 0)).astype(F16),
        ))

    xThilo = [_hilo(np.ascontiguousarray(x[b].T)) for b in range(B)]

    i = np.arange(P)[:, None]
    j = np.arange(P)[None, :]
    tri = np.ascontiguousarray((j >= i).astype(F16))

    for c in range(8):
        b, hg = c // 4, c % 4
        qs = slice(NQ * hg, NQ * (hg + 1))
        ks = slice(NKV * hg, NKV * (hg + 1))
        cos_t, sin_t = tables[b]
        xh, xl = xThilo[b]
        # wq8[h]: [P, 2(hi/lo), NDP, 2(pair-half), H], partition-major
        wq_h, wq_l = _hilo(SW_QK * wq[qs])                  # [8, D, H]
        wq8 = np.ascontiguousarray(
            np.stack([wq_h, wq_l], 1)
            .reshape(NQ, 2, NDP, 2, P, H).transpose(0, 4, 1, 2, 3, 5))
        wk_h, wk_l = _hilo(SW_QK * wkv[0, ks].transpose(1, 0, 2))
        wk8 = np.ascontiguousarray(
            np.stack([wk_h, wk_l], 0)
            .reshape(2, NDP, 2, P, NKV, H).transpose(3, 0, 1, 2, 4, 5))
        wv_h, wv_l = _hilo(SW_V * wkv[1, ks].transpose(1, 0, 2))
        wv8 = np.ascontiguousarray(
            np.stack([wv_h, wv_l], 0)
            .reshape(2, NDP, 2, P, NKV * H).transpose(3, 0, 1, 2, 4))
        # wo: [8, H, D] -> [H, 8, D]; adjacent heads form DoubleRow pairs
        wo_h, wo_l = _hilo((SW_O * wo[qs]).transpose(1, 0, 2))
        in_maps.append({
            "xhi": xh,
            "xlo": xl,
            "wq8": wq8,
            "wk8": wk8,
            "wv8": wv8,
            "woh": wo_h,
            "wol": wo_l,
            "cos": cos_t,
            "sin": sin_t,
            "tri": tri,
        })
    return in_maps


def run_cores(in_maps, trace=False, trace_cores=None):
    from concourse.bass_utils import run_bass_kernel_spmd
    nc = _get_nc()
    kw = {}
    if trace:
        kw = dict(trace=True,
                  trace_cores=trace_cores or list(range(8)))
    return run_bass_kernel_spmd(nc, in_maps, core_ids=list(range(8)), **kw)


def kernel(**inputs):
    x = np.asarray(inputs["x"], np.float32)
    positions = np.asarray(inputs["positions"])
    wq = np.asarray(inputs["wq"], np.float32)
    wkv = np.asarray(inputs["wkv"], np.float32)
    wo = np.asarray(inputs["wo"], np.float32)
    B = x.shape[0]
    assert x.shape == (2, T, D) and wq.shape == (32, D, H)

    in_maps = _make_in_maps(x, positions, wq, wkv, wo)
    res = run_cores(in_maps)
    y = np.zeros((B, T, D), np.float32)
    inv = 1.0 / (SW_V * SW_O)
    for c, r in enumerate(res.results):
        y[c // 4] += np.asarray(r["y"], np.float32) * inv
    return y


if __name__ == "__main__":
    _build_nc()
    print("build OK")
